# revision 12
# baseline (speedup 1.0000x reference)
"""Trainium2 Bass kernel for nn_Net_50620484551136 (gnn_message_passing).

Network (see problem reference):
  h  = MLP(x)                     # 4652 -> 256 -> 256
  h1 = relu(GCN(h, e1)); h2 = relu(GCN(h, e2))
  h  = MLP([h1, h2])              # 512 -> 256 -> 256
  h1 = relu(GCN(h, e1)); h2 = relu(GCN(h, e2))
  h  = MLP([h1, h2])
  r1 = scatter_mean(h, index_1, N); r2 = scatter_mean(h, index_2, N)
  out = log_softmax(MLP([r1, r2]))

Strategy (8 NeuronCores, SPMD single program):
  - Tuple nodes sharded contiguously across cores (6250/core, padded 6272).
  - Fully fused pipeline: per 4-tile group (512 nodes) of each round we run
    [gathers -> SEG aggregation -> conv -> MLP -> next-round table write] so
    the PE stays continuously busy (p-state!) and no h tensors bounce
    through DRAM.
  - GCN aggregation via PE matmuls against host-built one-hot SEG blocks in
    fp8e4 with perf_mode=DoubleRow (0.5 cycles/row): SEG carries
    dinv[src]*dinv[dst]*SEG_BOOST, gathered table h*G_SCALE in fp8e4, conv
    weights pre-divided by SEG_BOOST*G_SCALE so the natural scale returns.
  - Self-loop (diagonal) term added node-major from resident bf16 gt tiles:
    aggs = agg_psum + gt * (dinv^2 * SEG_BOOST * G_SCALE).
  - Gathers are grouped 4 tiles per dma_gather call (one lo + one hi call
    per group per relation; int16 index split at 32768) with runtime count
    registers trimming trailing pads.
  - Readout (scatter-mean) push-model binned into [5120, 256] with 1/count
    folded into fp8 SEG (x SEG_BOOST), one ReduceScatter(add) per index.
  - Input MLP layer 1 in fp8e4 DoubleRow: x cast to fp8e4, W_i1 * 1024 in
    fp8e4, descaled by 2^-10 in the relu-bias activation.
"""

import numpy as np
import ml_dtypes

BF16 = ml_dtypes.bfloat16
FP8E4 = ml_dtypes.float8_e4m3

# Problem constants (hardcoded per harness contract).
T = 50000
N_BINS = 5000
F_IN = 4652
DIM = 256
N_CLASSES = 5
NCORES = 8
SPLIT = 32768  # int16 gather index limit

SINGLE_PACKET = False
G_SCALE = 16.0       # gathered table stores h * G_SCALE in fp8e4
SEG_BOOST = 64.0     # SEG stores norm * SEG_BOOST in fp8e4
POST_SCALE = 1.0 / (G_SCALE * SEG_BOOST)  # 2^-10, folded into conv weights
FP8_X = True         # input MLP layer 1 in fp8e4 DoubleRow
GROUP = 4            # tiles per gather/conv/MLP group
RO_GROUP = 8         # bin-tiles per readout gather


def _ceil_to(x, m):
    return (x + m - 1) // m * m


def _even(x):
    return x + (x & 1)


def _wrap_idx(v):
    """int16 index vector (len % 16 == 0) -> [128, len/16] wrapped layout."""
    assert len(v) % 16 == 0
    w = v.reshape(-1, 16).T.astype(np.int16)  # [16, len/16]
    return np.tile(w, (8, 1))  # [128, len/16]


def _chunk_weight(w, dtype=BF16, kpad=None):
    """[K, M] -> [128, ceil(K/128), M] (partition = k%128, block = k//128)."""
    k, m = w.shape
    kp = kpad if kpad is not None else _ceil_to(k, 128)
    wp = np.zeros((kp, m), np.float32)
    wp[:k] = w
    return np.ascontiguousarray(
        wp.reshape(kp // 128, 128, m).transpose(1, 0, 2)
    ).astype(dtype)


def _chunk_bias(b):
    """[M] -> [128, ceil(M/128)] f32 (partition = m%128, col = m//128)."""
    m = len(b)
    mp = _ceil_to(m, 128)
    bp = np.zeros(mp, np.float32)
    bp[:m] = b
    return np.ascontiguousarray(bp.reshape(mp // 128, 128).T).astype(np.float32)


def _prep_rel(src, dst, vals, dpc, dpad, ncores, gpos, ngr):
    """Per-core grouped gather idx / SEG / counts for one edge relation.

    dst space sharded dpc per core (padded dpad, nt tiles, groups of GROUP
    tiles).  Source row in the gathered table is gpos[src]; vals[e] is the
    SEG weight (already boosted).  Per group one lo and one hi gather; tile
    j of a group occupies slots [j*nb_lo*128, ...) of the lo region and
    [4*nb_lo*128 + j*nb_hi*128, ...) of the hi region.  Interior pads use
    idx 0 (gathers row 0, annihilated by zero SEG); counts trim the tail.
    """
    nt = dpad // 128
    order = np.argsort(dst, kind="stable")
    src, dst, vals = src[order], dst[order], vals[order]
    core_of = dst // dpc
    gsrc = gpos[src]
    ldst = gpos[dst]

    per_core = []
    nb_lo = 2
    nb_hi = 2
    for p in range(ncores):
        sel = core_of == p
        sp = gsrc[sel]
        vv = vals[sel]
        ld = ldst[sel] - p * dpad
        tiles = []
        for t in range(nt):
            m = (ld // 128) == t
            st = sp[m]
            dd = (ld[m] - t * 128).astype(np.int64)
            va = vv[m]
            lo = st < SPLIT
            ol = np.argsort(st[lo], kind="stable")
            oh = np.argsort(st[~lo], kind="stable")
            tiles.append((st[lo][ol], st[~lo][oh] - SPLIT,
                          dd[lo][ol], dd[~lo][oh], va[lo][ol], va[~lo][oh]))
            nb_lo = max(nb_lo, _even(_ceil_to(max(len(st[lo]), 1), 128) // 128))
            nb_hi = max(nb_hi, _even(_ceil_to(max(len(st[~lo]), 1), 128) // 128))
        per_core.append(tiles)

    nb = nb_lo + nb_hi
    idx_arrs, seg_arrs, cnt_arrs = [], [], []
    for p in range(ncores):
        idx_a = np.full((ngr, 128, GROUP * nb * 8), -1, np.int16)
        seg_a = np.zeros((ngr, 128, GROUP * nb * 128), np.float32)
        cnt_a = np.zeros((ngr, 2), np.int32)
        for g in range(ngr):
            gsz = min(GROUP, nt - g * GROUP)
            li_lo = np.full(GROUP * nb_lo * 128, -1, np.int64)
            li_hi = np.full(GROUP * nb_hi * 128, -1, np.int64)
            for j in range(gsz):
                t = g * GROUP + j
                lo_gs, hi_gs, lo_dd, hi_dd, lo_va, hi_va = per_core[p][t]
                o_lo = j * nb_lo * 128
                li_lo[o_lo:o_lo + nb_lo * 128] = 0
                li_lo[o_lo:o_lo + len(lo_gs)] = lo_gs
                o_hi = j * nb_hi * 128
                li_hi[o_hi:o_hi + nb_hi * 128] = 0
                li_hi[o_hi:o_hi + len(hi_gs)] = hi_gs
                if j == gsz - 1:
                    cnt_a[g, 0] = o_lo + max(len(lo_gs), 1)
                    cnt_a[g, 1] = o_hi + max(len(hi_gs), 1)
                    li_lo[o_lo + max(len(lo_gs), 1):] = -1
                    li_hi[o_hi + max(len(hi_gs), 1):] = -1
                # SEG: lo blocks at (j*nb_lo + b), hi at (GROUP*nb_lo + j*nb_hi + b)
                i = np.arange(len(lo_dd)) + j * nb_lo * 128
                seg_a[g, i % 128,
                      ((i // 128) * 128 + lo_dd)] = lo_va
                i = np.arange(len(hi_dd)) + (GROUP * nb_lo + j * nb_hi) * 128
                seg_a[g, i % 128,
                      ((i // 128) * 128 + hi_dd)] = hi_va
            idx_a[g, :, :GROUP * nb_lo * 8] = _wrap_idx(li_lo.astype(np.int16))
            idx_a[g, :, GROUP * nb_lo * 8:] = _wrap_idx(li_hi.astype(np.int16))
        idx_arrs.append(idx_a)
        seg_arrs.append(np.ascontiguousarray(seg_a.astype(FP8E4)))
        cnt_arrs.append(cnt_a)
    return dict(nb_lo=nb_lo, nb_hi=nb_hi, idx=idx_arrs, seg=seg_arrs,
                cnt=cnt_arrs)


def host_prep(inputs, ncores=NCORES, n_bins=None):
    """Pure-numpy preprocessing: sharding, edge sorting, SEG/idx/count
    construction, weight and x layout."""
    x = np.asarray(inputs["x"], np.float32)
    t_nodes, f_in = x.shape
    dim = np.asarray(inputs["W_i2"]).shape[0]
    ncls = np.asarray(inputs["b_fb"]).shape[0]
    if n_bins is None:
        if t_nodes == T and f_in == F_IN:
            n_bins = N_BINS
        else:
            n_bins = int(np.asarray(inputs["index_1"]).max()) + 1

    assert t_nodes % ncores == 0, (t_nodes, ncores)
    tpc = t_nodes // ncores
    tpad = _ceil_to(tpc, 128)
    nt = tpad // 128
    ngr = _ceil_to(nt, GROUP) // GROUP
    kin = _ceil_to(f_in, 256)  # even number of 128-blocks for DoubleRow
    assert n_bins % ncores == 0, (n_bins, ncores)
    bpc = n_bins // ncores
    bpad = _ceil_to(bpc, 128)
    bt = bpad // 128            # tiles per core's bin shard
    btg = ncores * bt           # global padded bin tiles
    n_rogr = _ceil_to(btg, RO_GROUP) // RO_GROUP

    cfg = dict(
        t_nodes=t_nodes, f_in=f_in, dim=dim, ncls=ncls, n_bins=n_bins,
        ncores=ncores, tpc=tpc, tpad=tpad, nt=nt, ngr=ngr,
        kin=kin, kc=kin // 128,
        bpc=bpc, bpad=bpad, bt=bt, btg=btg, n_rogr=n_rogr,
        g_rows=ncores * tpad,
    )

    # ---- conv relations: drop self-loops via norm folding
    edges = {}
    for r, key in ((1, "edge_index_1"), (2, "edge_index_2")):
        ei = np.asarray(inputs[key]).astype(np.int64)
        s, d = ei[0], ei[1]
        deg = np.bincount(d, minlength=t_nodes).astype(np.float64) + 1.0
        dinv = (1.0 / np.sqrt(deg)).astype(np.float32)
        edges[r] = (s, d, dinv)

    # ---- per-core node permutation balancing per-tile gather-slot counts.
    straddle = SPLIT // tpad  # core whose row range contains SPLIT

    def core_deg4(p, inv_s):
        deg4 = np.zeros((tpc, 4), np.int64)
        for ci, r in enumerate((1, 2)):
            s, d, _ = edges[r]
            sel = (d // tpc) == p
            sl, dl = s[sel], d[sel] - p * tpc
            sc = sl // tpc
            srow = np.where(
                sc == straddle,
                straddle * tpad + inv_s[np.minimum(
                    np.maximum(sl - straddle * tpc, 0), tpc - 1)],
                sc * tpad + (sl % tpc))
            is_lo = srow < SPLIT
            np.add.at(deg4[:, 2 * ci], dl[is_lo], 1)
            np.add.at(deg4[:, 2 * ci + 1], dl[~is_lo], 1)
        return deg4

    ident = np.arange(tpc, dtype=np.int64)
    inv_s = ident  # pass 1: approximate straddle-core positions
    for _ in range(2):
        perm_s = _balance_perm(core_deg4(straddle, inv_s), nt)
        inv_s = np.empty(tpc, np.int64)
        inv_s[perm_s] = ident

    perms = []
    gpos = np.empty(t_nodes, np.int64)
    for p in range(ncores):
        if p == straddle:
            perm = perm_s
        else:
            perm = _balance_perm(core_deg4(p, inv_s), nt)
        perms.append(perm)
        inv = np.empty(tpc, np.int64)
        inv[perm] = ident
        gpos[p * tpc: (p + 1) * tpc] = p * tpad + inv
    cfg["perms"] = perms

    rel = {}
    for r in (1, 2):
        s, d, dinv = edges[r]
        vals = dinv[s] * dinv[d] * SEG_BOOST
        rel[r] = dict(
            prep=_prep_rel(s, d, vals, tpc, tpad, ncores, gpos, ngr),
            dinv=dinv,
        )
    cfg["rel"] = rel

    # ---- readout: push-model over local nodes into global padded bin rows
    ro = {}
    for i, key in ((1, "index_1"), (2, "index_2")):
        idx = np.asarray(inputs[key]).astype(np.int64)
        cnt = np.bincount(idx, minlength=n_bins).astype(np.float64)
        invc = (1.0 / np.maximum(cnt, 1.0)).astype(np.float32) * SEG_BOOST
        grow = (idx // bpc) * bpad + (idx % bpc)  # padded global bin row
        nbro = 2
        percore = []
        for p in range(ncores):
            pm = cfg["perms"][p]
            n_loc = np.arange(tpc, dtype=np.int64)
            g = grow[p * tpc: (p + 1) * tpc][pm]
            v = invc[idx[p * tpc: (p + 1) * tpc][pm]]
            tiles = []
            for tT in range(btg):
                m = (g // 128) == tT
                nn = n_loc[m]
                dd = (g[m] - tT * 128).astype(np.int64)
                vv = v[m]
                o = np.argsort(nn, kind="stable")
                tiles.append((nn[o], dd[o], vv[o]))
                nbro = max(nbro,
                           _even(_ceil_to(max(len(nn), 1), 128) // 128))
            percore.append(tiles)
        idx_arrs, seg_arrs, cnt_arrs = [], [], []
        for p in range(ncores):
            idx_a = np.full((n_rogr, 128, RO_GROUP * nbro * 8), -1, np.int16)
            seg_a = np.zeros((n_rogr, 128, RO_GROUP * nbro * 128), np.float32)
            cnt_a = np.zeros(n_rogr, np.int32)
            for g in range(n_rogr):
                gsz = min(RO_GROUP, btg - g * RO_GROUP)
                li = np.full(RO_GROUP * nbro * 128, -1, np.int64)
                for j in range(gsz):
                    tT = g * RO_GROUP + j
                    nn, dd, vv = percore[p][tT]
                    o = j * nbro * 128
                    li[o:o + nbro * 128] = 0
                    li[o:o + len(nn)] = nn
                    if j == gsz - 1:
                        cnt_a[g] = o + max(len(nn), 1)
                        li[o + max(len(nn), 1):] = -1
                    k = np.arange(len(dd)) + o
                    seg_a[g, k % 128, (k // 128) * 128 + dd] = vv
                idx_a[g] = _wrap_idx(li.astype(np.int16))
            idx_arrs.append(idx_a)
            seg_arrs.append(np.ascontiguousarray(seg_a.astype(FP8E4)))
            cnt_arrs.append(cnt_a)
        ro[i] = dict(prep=dict(nb=nbro, idx=idx_arrs, seg=seg_arrs,
                               cnt=cnt_arrs))
    cfg["ro"] = ro

    # ---- counts tensor per core: [128, CNT_COLS] int32 (replicated rows)
    # layout: rel1 (ngr*2: lo,hi), rel2 (ngr*2), ro1 (n_rogr), ro2 (n_rogr)
    cnt_cols = 2 * ngr * 2 + 2 * n_rogr
    cfg["cnt_cols"] = cnt_cols
    cnts = []
    for p in range(ncores):
        c = np.concatenate([
            rel[1]["prep"]["cnt"][p].reshape(-1),
            rel[2]["prep"]["cnt"][p].reshape(-1),
            ro[1]["prep"]["cnt"][p],
            ro[2]["prep"]["cnt"][p],
        ]).astype(np.int32)
        assert len(c) == cnt_cols
        cnts.append(np.ascontiguousarray(np.tile(c[None, :], (128, 1))))
    cfg["cnts"] = cnts

    # ---- per-core x in chunked layout [NCHUNK, 128, kc*512]
    nch = _ceil_to(tpad, 512) // 512
    cfg["nch"] = nch
    kc = kin // 128
    xdt = FP8E4 if FP8_X else BF16
    xTc = []
    for p in range(ncores):
        xs = np.zeros((kin, nch * 512), np.float32)
        xs[:f_in, :tpc] = x[p * tpc: (p + 1) * tpc][cfg["perms"][p]].T
        a = np.ascontiguousarray(
            xs.reshape(kc, 128, nch, 512).transpose(2, 1, 0, 3)
            .reshape(nch, 128, kc * 512)
        ).astype(xdt)
        xTc.append(a)
    cfg["xTc"] = xTc

    # ---- dinv^2 node-major [128, nt] f32 per relation per core
    # (partition = node slot within tile; value dinv^2 * SEG_BOOST * G_SCALE
    #  so that gt(bf16, natural h) * this == 1024 * h * dinv^2)
    for r in (1, 2):
        dn = []
        dinv2 = rel[r]["dinv"] ** 2 * (SEG_BOOST * G_SCALE)
        for p in range(ncores):
            vp = np.zeros(tpad, np.float32)
            vp[:tpc] = dinv2[p * tpc: (p + 1) * tpc][cfg["perms"][p]]
            dn.append(np.ascontiguousarray(
                vp.reshape(nt, 128).T.astype(np.float32)))
        rel[r]["dinv2_nm"] = dn

    # ---- weights
    w = {}
    if FP8_X:
        w["wi1"] = _chunk_weight(
            np.asarray(inputs["W_i1"], np.float32) * (G_SCALE * SEG_BOOST),
            FP8E4, kpad=kin)
    else:
        w["wi1"] = _chunk_weight(np.asarray(inputs["W_i1"], np.float32),
                                 kpad=kin)
    w["wi2"] = _chunk_weight(np.asarray(inputs["W_i2"], np.float32))
    for nm, src in (("wc11", "Wc11"), ("wc12", "Wc12"),
                    ("wc21", "Wc21"), ("wc22", "Wc22")):
        w[nm] = _chunk_weight(np.asarray(inputs[src], np.float32) * POST_SCALE)
    for nm, src in (("wm1a", "W_m1a"), ("wm1b", "W_m1b"),
                    ("wm2a", "W_m2a"), ("wm2b", "W_m2b"),
                    ("wfa", "W_fa"), ("wfb", "W_fb")):
        w[nm] = _chunk_weight(np.asarray(inputs[src], np.float32))
    for nm, src in (("bi1", "b_i1"), ("bi2", "b_i2"),
                    ("bc11", "bc11"), ("bc12", "bc12"),
                    ("bc21", "bc21"), ("bc22", "bc22"),
                    ("bm1a", "b_m1a"), ("bm1b", "b_m1b"),
                    ("bm2a", "b_m2a"), ("bm2b", "b_m2b"),
                    ("bfa", "b_fa"), ("bfb", "b_fb")):
        w[nm] = _chunk_bias(np.asarray(inputs[src], np.float32))
    w["ident16"] = np.eye(128, dtype=BF16)
    w["ident32"] = np.eye(128, dtype=np.float32)
    cfg["w"] = w
    return cfg


def _balance_perm(deg4, nt, cap=128):
    """Greedy assignment of nodes to tiles balancing 4 degree components."""
    n_nodes = deg4.shape[0]
    order = np.argsort(-deg4.sum(1), kind="stable")
    loads = np.zeros((nt, 4))
    counts = np.zeros(nt, np.int64)
    capv = np.full(nt, cap, np.int64)
    capv[-1] = n_nodes - (nt - 1) * cap
    wscale = 1.0 / np.maximum(deg4.mean(0), 1e-9)
    tiles = [[] for _ in range(nt)]
    for n in order:
        avail = np.nonzero(counts < capv)[0]
        after = ((loads[avail] + deg4[n]) * wscale).max(1)
        j = avail[np.argmin(after + 1e-6 * loads[avail].sum(1))]
        tiles[j].append(n)
        loads[j] += deg4[n]
        counts[j] += 1
    perm = np.empty(n_nodes, np.int64)
    for t in range(nt):
        sl = np.sort(np.array(tiles[t], np.int64))
        perm[t * cap: t * cap + len(sl)] = sl
    return perm


def _nchunks(total, step):
    out = []
    o = 0
    while o < total:
        out.append((o, min(step, total - o)))
        o += step
    return out


def build_program(cfg):
    """Build the SPMD bass program (one program, 8 cores)."""
    import concourse.bass as bass
    import concourse.mybir as mybir
    import concourse.tile as tile
    from concourse import bacc

    dt = mybir.dt
    AF = mybir.ActivationFunctionType
    ALU = mybir.AluOpType
    DR = mybir.MatmulPerfMode.DoubleRow

    nt, tpad, kc = cfg["nt"], cfg["tpad"], cfg["kc"]
    ngr = cfg["ngr"]
    bt, bpad, btg = cfg["bt"], cfg["bpad"], cfg["btg"]
    n_rogr = cfg["n_rogr"]
    dim, ncls = cfg["dim"], cfg["ncls"]
    dc = dim // 128
    g_rows = cfg["g_rows"]
    ncores = cfg["ncores"]
    nch = cfg["nch"]
    rel, ro = cfg["rel"], cfg["ro"]
    rg = [list(range(ncores))]

    nb_r = {r: rel[r]["prep"]["nb_lo"] + rel[r]["prep"]["nb_hi"]
            for r in (1, 2)}
    nbro = {i: ro[i]["prep"]["nb"] for i in (1, 2)}
    # shared ed/seg/idx pool block capacity
    nb_max = max(max(GROUP * nb_r[r] for r in (1, 2)),
                 max(RO_GROUP * nbro[i] for i in (1, 2)))

    GDT = dt.float8e4
    XDT = dt.float8e4 if FP8_X else dt.bfloat16

    nc = bacc.Bacc("TRN2", target_bir_lowering=False, debug=False,
                   num_devices=ncores, num_swdge_queues=4)
    qstate = [0]

    def next_q():
        q = qstate[0]
        qstate[0] = (q + 1) % 4
        return q

    # ---------------- I/O declarations ----------------
    xTc = nc.dram_tensor("xTc", [nch, 128, kc * 512], XDT,
                         kind="ExternalInput")
    seg_in, idx_in, dinv2_in = {}, {}, {}
    for r in (1, 2):
        nb = nb_r[r]
        seg_in[r] = nc.dram_tensor(f"seg{r}", [ngr, 128, GROUP * nb * 128],
                                   dt.float8e4, kind="ExternalInput")
        idx_in[r] = nc.dram_tensor(f"idx{r}", [ngr, 128, GROUP * nb * 8],
                                   dt.int16, kind="ExternalInput")
        dinv2_in[r] = nc.dram_tensor(f"dinv2nm{r}", [128, nt], dt.float32,
                                     kind="ExternalInput")
    segr_in, idxr_in = {}, {}
    for i in (1, 2):
        nb = nbro[i]
        segr_in[i] = nc.dram_tensor(f"segr{i}",
                                    [n_rogr, 128, RO_GROUP * nb * 128],
                                    dt.float8e4, kind="ExternalInput")
        idxr_in[i] = nc.dram_tensor(f"idxr{i}",
                                    [n_rogr, 128, RO_GROUP * nb * 8],
                                    dt.int16, kind="ExternalInput")
    cnts_in = nc.dram_tensor("cnts", [128, cfg["cnt_cols"]], dt.int32,
                             kind="ExternalInput")

    wnames_w = dict(
        wi2=[128, dc, dim],
        wc11=[128, dc, dim], wc12=[128, dc, dim],
        wc21=[128, dc, dim], wc22=[128, dc, dim],
        wm1a=[128, 2 * dc, dim], wm1b=[128, dc, dim],
        wm2a=[128, 2 * dc, dim], wm2b=[128, dc, dim],
        wfa=[128, 2 * dc, dim], wfb=[128, dc, ncls],
        ident16=[128, 128],
    )
    wnames_f32 = dict(
        bi1=[128, dc], bi2=[128, dc],
        bc11=[128, dc], bc12=[128, dc], bc21=[128, dc], bc22=[128, dc],
        bm1a=[128, dc], bm1b=[128, dc], bm2a=[128, dc], bm2b=[128, dc],
        bfa=[128, dc], bfb=[128, 1],
        ident32=[128, 128],
    )
    win = {}
    win["wi1"] = nc.dram_tensor("wi1", [128, kc, dim], XDT,
                                kind="ExternalInput")
    for nm, shp in wnames_w.items():
        win[nm] = nc.dram_tensor(nm, shp, dt.bfloat16, kind="ExternalInput")
    for nm, shp in wnames_f32.items():
        win[nm] = nc.dram_tensor(nm, shp, dt.float32, kind="ExternalInput")

    out_dram = nc.dram_tensor("out", [bpad, ncls], dt.float32,
                              kind="ExternalOutput")

    # counts column offsets
    def cnt_col_rel(r, g, hi):
        return (r - 1) * ngr * 2 + g * 2 + (1 if hi else 0)

    def cnt_col_ro(i, g):
        return 2 * ngr * 2 + (i - 1) * n_rogr + g

    from contextlib import ExitStack

    with tile.TileContext(nc) as tc, ExitStack() as es:
        if True:
            wpool = es.enter_context(tc.tile_pool(name="wpool", bufs=1))
            xpool = es.enter_context(tc.tile_pool(name="xpool", bufs=2))
            hstgp = es.enter_context(tc.tile_pool(name="hstg", bufs=2))
            houtp = es.enter_context(tc.tile_pool(name="hout", bufs=2))
            aggtp = es.enter_context(tc.tile_pool(name="aggtp", bufs=2))
            apool = es.enter_context(tc.tile_pool(name="apool", bufs=3))
            edpool = es.enter_context(tc.tile_pool(name="edpool", bufs=4))
            segpool = es.enter_context(tc.tile_pool(name="segpool", bufs=4))
            idxpool = es.enter_context(tc.tile_pool(name="idxpool", bufs=5))
            gtpool = es.enter_context(tc.tile_pool(name="gtpool", bufs=1))
            g8pool = es.enter_context(tc.tile_pool(name="g8pool", bufs=2))
            mpool = es.enter_context(tc.tile_pool(name="mpool", bufs=4))
            pbig = es.enter_context(
                tc.tile_pool(name="pbig", bufs=3, space="PSUM"))
            pps = es.enter_context(
                tc.tile_pool(name="pps", bufs=3, space="PSUM"))
            pcnv = es.enter_context(
                tc.tile_pool(name="pcnv", bufs=2, space="PSUM"))
            dpool = es.enter_context(
                tc.tile_pool(name="dpool", bufs=1, space="DRAM"))
            # ---- resident weights + counts
            wsb = {}
            wt = wpool.tile([128, kc, dim], XDT, name="sb_wi1", tag="w_wi1")
            nc.sync.dma_start(wt[:], win["wi1"][:])
            wsb["wi1"] = wt
            for nm in list(wnames_w) + list(wnames_f32):
                shp = wnames_w.get(nm) or wnames_f32[nm]
                dtyp = dt.bfloat16 if nm in wnames_w else dt.float32
                wt = wpool.tile(shp, dtyp, name=f"sb_{nm}", tag=f"w_{nm}")
                nc.sync.dma_start(wt[:], win[nm][:])
                wsb[nm] = wt
            dinv2_sb = {}
            for r in (1, 2):
                dv = wpool.tile([128, nt], dt.float32, name=f"sb_dinv2{r}",
                                tag=f"w_dinv2{r}")
                nc.sync.dma_start(dv[:], dinv2_in[r][:])
                dinv2_sb[r] = dv
            cnts_sb = wpool.tile([128, cfg["cnt_cols"]], dt.int32,
                                 name="sb_cnts", tag="w_cnts")
            nc.sync.dma_start(cnts_sb[:], cnts_in[:])
            cregs = [nc.gpsimd.alloc_register(f"gather_cnt{q}")
                     for q in range(4)]

            # zero the gather destination pool once so skipped (padded) slots
            # always hold finite stale values (SEG zeros annihilate them)
            for b in range(4):
                edt = edpool.tile([128, nb_max, dim], GDT, name="ed", tag="ed")
                nc.vector.memset(edt[:], 0.0)

            # resident bf16 node-major h tiles (diag term source)
            gt_bf = [gtpool.tile([128, dim], dt.bfloat16, name=f"gtb{t}",
                                 tag=f"gtb{t}") for t in range(nt)]

            g_loc, g_full = {}, {}
            for rnd in (1, 2):
                g_loc[rnd] = dpool.tile([tpad, dim], GDT, name=f"g_loc{rnd}",
                                        tag=f"g_loc{rnd}")
                g_full[rnd] = dpool.tile([g_rows, dim], GDT,
                                         name=f"g_full{rnd}",
                                         tag=f"g_full{rnd}",
                                         addr_space="Shared")
            hf_loc = dpool.tile([tpad, dim], GDT, name="hf_loc", tag="hf_loc")

            def table_write(hstg, g, gsz, dst):
                """Transpose feature-major hstg [128, dc, 512] chunk into
                node-major bf16 gt tiles + fp8 rows of dst (x G_SCALE)."""
                for j in range(gsz):
                    t = g * GROUP + j
                    g8 = g8pool.tile([128, dim], GDT, name="g8", tag="g8")
                    for f in range(dc):
                        tp = pps.tile([128, 128], dt.bfloat16, name="tw",
                                       tag="ps")
                        nc.tensor.transpose(
                            tp[:], hstg[:, f, j * 128:(j + 1) * 128],
                            wsb["ident16"][:])
                        nc.vector.tensor_copy(
                            gt_bf[t][:, f * 128:(f + 1) * 128], tp[:])
                        nc.scalar.activation(
                            g8[:, f * 128:(f + 1) * 128], tp[:], AF.Copy,
                            scale=G_SCALE)
                    nc.sync.dma_start(dst[t * 128:(t + 1) * 128, :], g8[:])

            # =========== Phase 1: input MLP  h0 = relu(x@Wi1+bi1)@Wi2+bi2
            kgs = _nchunks(kc, 8)  # k-groups of 8 k-blocks (all even)
            for g in range(ngr):
                gsz = min(GROUP, nt - g * GROUP)
                n0 = g * 512
                nw = gsz * 128
                ps1 = [pbig.tile([128, 512], dt.float32, name=f"ps1_{f}",
                                 tag="mlp") for f in range(dc)]
                for (k0, kw) in kgs:
                    xg = xpool.tile([128, 8, 512], XDT, name="xg", tag="xg")
                    nc.sync.dma_start(
                        xg[:, :kw, :],
                        xTc[g, :, k0 * 512: (k0 + kw) * 512])
                    if FP8_X:
                        for ki in range(0, kw, 2):
                            k = k0 + ki
                            for f in range(dc):
                                for h0 in range(0, nw, 256):
                                    hw_ = min(256, nw - h0)
                                    nc.tensor.matmul(
                                        ps1[f][:, h0:h0 + hw_],
                                        lhsT=wsb["wi1"][:, k:k + 2,
                                                        f * 128:(f + 1) * 128],
                                        rhs=xg[:, ki:ki + 2, h0:h0 + hw_],
                                        start=(k == 0), stop=(k == kc - 2),
                                        perf_mode=DR)
                    else:
                        for ki in range(kw):
                            k = k0 + ki
                            for f in range(dc):
                                nc.tensor.matmul(
                                    ps1[f][:, :nw],
                                    lhsT=wsb["wi1"][:, k,
                                                    f * 128:(f + 1) * 128],
                                    rhs=xg[:, ki, :nw],
                                    start=(k == 0), stop=(k == kc - 1))
                a1 = []
                for f in range(dc):
                    a_ = apool.tile([128, 512], dt.bfloat16, name=f"a1_{f}",
                                    tag="a1")
                    nc.scalar.activation(
                        a_[:, :nw], ps1[f][:, :nw], AF.Relu,
                        bias=wsb["bi1"][:, f:f + 1],
                        scale=(POST_SCALE if FP8_X else 1.0))
                    a1.append(a_)
                hstg = hstgp.tile([128, dc, 512], dt.bfloat16, name="h0s",
                                  tag="hstg")
                for f2 in range(dc):
                    p2 = pbig.tile([128, 512], dt.float32, name="ps2",
                                   tag="mlp")
                    for k2 in range(dc):
                        nc.tensor.matmul(
                            p2[:, :nw],
                            lhsT=wsb["wi2"][:, k2, f2 * 128:(f2 + 1) * 128],
                            rhs=a1[k2][:, :nw],
                            start=(k2 == 0), stop=(k2 == dc - 1))
                    nc.vector.tensor_scalar(
                        hstg[:, f2, :nw], p2[:, :nw],
                        wsb["bi2"][:, f2:f2 + 1], None, ALU.add)
                table_write(hstg, g, gsz, g_loc[1][:])
            nc.gpsimd.collective_compute(
                "AllGather", ALU.bypass, replica_groups=rg,
                ins=[g_loc[1][:]], outs=[g_full[1][:]])

            # =========== Phase 2: two fused GCN+MLP rounds
            for rnd in (1, 2):
                wma = wsb[f"wm{rnd}a"]
                wmb = wsb[f"wm{rnd}b"]
                bma = wsb[f"bm{rnd}a"]
                bmb = wsb[f"bm{rnd}b"]
                for g in range(ngr):
                    gsz = min(GROUP, nt - g * GROUP)
                    nw = gsz * 128
                    eds, segs, idxs = {}, {}, {}
                    for r in (1, 2):
                        nb = nb_r[r]
                        idxt = idxpool.tile([128, nb_max * 8], dt.int16,
                                            name="idxt", tag="idx")
                        nc.sync.dma_start(idxt[:, :GROUP * nb * 8],
                                          idx_in[r][g])
                        idxs[r] = idxt
                        segt = segpool.tile([128, nb_max, 128], dt.float8e4,
                                            name="segt", tag="seg")
                        nc.scalar.dma_start(segt[:, :GROUP * nb, :],
                                            seg_in[r][g])
                        segs[r] = segt
                    for r in (1, 2):
                        pr = rel[r]["prep"]
                        nb_lo, nb_hi = pr["nb_lo"], pr["nb_hi"]
                        nb = nb_lo + nb_hi
                        idxt = idxs[r]
                        ed = edpool.tile([128, nb_max, dim], GDT,
                                         name="ed", tag="ed")
                        q = next_q()
                        nc.gpsimd.reg_load(
                            cregs[q], cnts_sb[0:1, cnt_col_rel(r, g, False):
                                              cnt_col_rel(r, g, False) + 1])
                        nc.gpsimd.dma_gather(
                            ed[:, 0:GROUP * nb_lo, :], g_full[rnd][:],
                            idxt[:, 0:GROUP * nb_lo * 8],
                            GROUP * nb_lo * 128, cregs[q], dim,
                            single_packet=SINGLE_PACKET, queue_num=q)
                        q = next_q()
                        nc.gpsimd.reg_load(
                            cregs[q], cnts_sb[0:1, cnt_col_rel(r, g, True):
                                              cnt_col_rel(r, g, True) + 1])
                        nc.gpsimd.dma_gather(
                            ed[:, GROUP * nb_lo:GROUP * nb, :],
                            g_full[rnd][SPLIT:g_rows, :],
                            idxt[:, GROUP * nb_lo * 8:GROUP * nb * 8],
                            GROUP * nb_hi * 128, cregs[q], dim,
                            single_packet=SINGLE_PACKET, queue_num=q)
                        eds[r] = ed

                    aggT = {r: aggtp.tile([128, dc, 512], dt.bfloat16,
                                          name=f"aggT{r}", tag=f"aggT{r}")
                            for r in (1, 2)}
                    for j in range(gsz):
                        t = g * GROUP + j
                        for r in (1, 2):
                            pr = rel[r]["prep"]
                            nb_lo, nb_hi = pr["nb_lo"], pr["nb_hi"]
                            ed, segt = eds[r], segs[r]
                            agg = pps.tile([128, dim], dt.float32,
                                            name="agg", tag="ps")
                            npair = (nb_lo + nb_hi) // 2
                            bi = 0
                            for bp in range(nb_lo // 2):
                                off = j * nb_lo + 2 * bp
                                nc.tensor.matmul(
                                    agg[:],
                                    lhsT=segt[:, off:off + 2, :],
                                    rhs=ed[:, off:off + 2, :],
                                    start=(bi == 0), stop=(bi == npair - 1),
                                    perf_mode=DR)
                                bi += 1
                            for bp in range(nb_hi // 2):
                                off = GROUP * nb_lo + j * nb_hi + 2 * bp
                                nc.tensor.matmul(
                                    agg[:],
                                    lhsT=segt[:, off:off + 2, :],
                                    rhs=ed[:, off:off + 2, :],
                                    start=(bi == 0), stop=(bi == npair - 1),
                                    perf_mode=DR)
                                bi += 1
                            # diagonal term (node-major): gt * dinv2*1024
                            tmpd = mpool.tile([128, dim], dt.bfloat16,
                                              name="tmpd", tag="tmpd")
                            nc.vector.tensor_scalar(
                                tmpd[:], gt_bf[t][:],
                                dinv2_sb[r][:, t:t + 1], None, ALU.mult)
                            aggs = mpool.tile([128, dim], dt.bfloat16,
                                              name="aggs", tag="aggs")
                            nc.vector.tensor_tensor(
                                aggs[:], agg[:], tmpd[:], ALU.add)
                            for f in range(dc):
                                tp = pps.tile([128, 128], dt.bfloat16,
                                               name="tpc", tag="ps")
                                nc.tensor.transpose(
                                    tp[:], aggs[:, f * 128:(f + 1) * 128],
                                    wsb["ident16"][:])
                                nc.vector.tensor_copy(
                                    aggT[r][:, f, j * 128:(j + 1) * 128],
                                    tp[:])
                    # conv (batched over the group)
                    hout = {}
                    for r in (1, 2):
                        wc = wsb[f"wc{rnd}{r}"]
                        bc = wsb[f"bc{rnd}{r}"]
                        ho = houtp.tile([128, dc, 512], dt.bfloat16,
                                        name=f"ho{r}", tag=f"ho{r}")
                        for f2 in range(dc):
                            pc = pcnv.tile([128, 512], dt.float32,
                                           name="pc", tag="cnv")
                            for k in range(dc):
                                nc.tensor.matmul(
                                    pc[:, :nw],
                                    lhsT=wc[:, k, f2 * 128:(f2 + 1) * 128],
                                    rhs=aggT[r][:, k, :nw],
                                    start=(k == 0), stop=(k == dc - 1))
                            nc.vector.tensor_scalar(
                                ho[:, f2, :nw], pc[:, :nw],
                                bc[:, f2:f2 + 1], 0.0, ALU.add, ALU.max)
                        hout[r] = ho
                    # MLP on concat(h1, h2) for this group's nodes
                    ps1 = [pbig.tile([128, 512], dt.float32, name="psm1",
                                     tag="mlp") for f in range(dc)]
                    for k in range(2 * dc):
                        rhs_t = hout[1] if k < dc else hout[2]
                        for f in range(dc):
                            nc.tensor.matmul(
                                ps1[f][:, :nw],
                                lhsT=wma[:, k, f * 128:(f + 1) * 128],
                                rhs=rhs_t[:, k % dc, :nw],
                                start=(k == 0), stop=(k == 2 * dc - 1))
                    am = []
                    for f in range(dc):
                        a_ = apool.tile([128, 512], dt.bfloat16, name="am",
                                        tag="a1")
                        nc.scalar.activation(a_[:, :nw], ps1[f][:, :nw],
                                             AF.Relu, bias=bma[:, f:f + 1])
                        am.append(a_)
                    hstg = hstgp.tile([128, dc, 512], dt.bfloat16,
                                      name="hms", tag="hstg")
                    for f2 in range(dc):
                        p2 = pbig.tile([128, 512], dt.float32, name="psm2",
                                       tag="mlp")
                        for k2 in range(dc):
                            nc.tensor.matmul(
                                p2[:, :nw],
                                lhsT=wmb[:, k2, f2 * 128:(f2 + 1) * 128],
                                rhs=am[k2][:, :nw],
                                start=(k2 == 0), stop=(k2 == dc - 1))
                        nc.vector.tensor_scalar(
                            hstg[:, f2, :nw], p2[:, :nw],
                            bmb[:, f2:f2 + 1], None, ALU.add)
                    # table write for the next stage
                    dst = g_loc[2][:] if rnd == 1 else hf_loc[:]
                    table_write(hstg, g, gsz, dst)
                if rnd == 1:
                    nc.gpsimd.collective_compute(
                        "AllGather", ALU.bypass, replica_groups=rg,
                        ins=[g_loc[2][:]], outs=[g_full[2][:]])

            # =========== Phase 3: readout (push + ReduceScatter)
            parts = {}
            for i in (1, 2):
                pr = ro[i]["prep"]
                nb = pr["nb"]
                part = dpool.tile([ncores * bpad, dim], dt.bfloat16,
                                  name=f"part{i}", tag=f"part{i}")
                for g in range(n_rogr):
                    gsz = min(RO_GROUP, btg - g * RO_GROUP)
                    idxt = idxpool.tile([128, nb_max * 8], dt.int16,
                                        name="idxtr", tag="idx")
                    nc.sync.dma_start(idxt[:, :RO_GROUP * nb * 8],
                                      idxr_in[i][g])
                    segt = segpool.tile([128, nb_max, 128], dt.float8e4,
                                        name="segtr", tag="seg")
                    nc.scalar.dma_start(segt[:, :RO_GROUP * nb, :],
                                        segr_in[i][g])
                    ed = edpool.tile([128, nb_max, dim], GDT,
                                     name="edr", tag="ed")
                    q = next_q()
                    nc.gpsimd.reg_load(
                        cregs[q], cnts_sb[0:1, cnt_col_ro(i, g):
                                          cnt_col_ro(i, g) + 1])
                    nc.gpsimd.dma_gather(
                        ed[:, 0:RO_GROUP * nb, :], hf_loc[:],
                        idxt[:, 0:RO_GROUP * nb * 8],
                        RO_GROUP * nb * 128, cregs[q], dim,
                        single_packet=SINGLE_PACKET, queue_num=q)
                    for j in range(gsz):
                        tT = g * RO_GROUP + j
                        agg = pps.tile([128, dim], dt.float32, name="aggr",
                                        tag="ps")
                        npair = nb // 2
                        for bp in range(npair):
                            off = j * nb + 2 * bp
                            nc.tensor.matmul(
                                agg[:],
                                lhsT=segt[:, off:off + 2, :],
                                rhs=ed[:, off:off + 2, :],
                                start=(bp == 0), stop=(bp == npair - 1),
                                perf_mode=DR)
                        aggs = mpool.tile([128, dim], dt.bfloat16,
                                          name="aggsr", tag="aggs")
                        nc.scalar.activation(aggs[:], agg[:], AF.Copy,
                                             scale=POST_SCALE)
                        nc.sync.dma_start(
                            part[tT * 128:(tT + 1) * 128, :], aggs[:])
                parts[i] = part
            rsh = {}
            for i in (1, 2):
                rs = dpool.tile([bpad, dim], dt.bfloat16, name=f"rsh{i}",
                                tag=f"rsh{i}")
                nc.gpsimd.collective_compute(
                    "ReduceScatter", ALU.add, replica_groups=rg,
                    ins=[parts[i][:]], outs=[rs[:]])
                rsh[i] = rs

            # transpose RS shards to feature-major rcat [128, 2*dc, bpad]
            rcat = wpool.tile([128, 2 * dc, bpad], dt.bfloat16, name="rcat",
                              tag="rcat")
            for i in (1, 2):
                for tb in range(bt):
                    rt = mpool.tile([128, dim], dt.bfloat16, name="rt",
                                    tag="rt")
                    nc.sync.dma_start(rt[:],
                                      rsh[i][tb * 128:(tb + 1) * 128, :])
                    for f in range(dc):
                        tp = pps.tile([128, 128], dt.bfloat16, name="tpr",
                                       tag="ps")
                        nc.tensor.transpose(
                            tp[:], rt[:, f * 128:(f + 1) * 128],
                            wsb["ident16"][:])
                        nc.vector.tensor_copy(
                            rcat[:, (i - 1) * dc + f,
                                 tb * 128:(tb + 1) * 128], tp[:])

            # ---- final MLP + log_softmax
            logitsT = wpool.tile([128, bpad], dt.float32, name="logitsT",
                                 tag="logitsT")
            for (n0, nw) in _nchunks(bpad, 512):
                ps1 = [pbig.tile([128, 512], dt.float32, name="psf1",
                                 tag="mlp") for f in range(dc)]
                for k in range(2 * dc):
                    for f in range(dc):
                        nc.tensor.matmul(
                            ps1[f][:, :nw],
                            lhsT=wsb["wfa"][:, k, f * 128:(f + 1) * 128],
                            rhs=rcat[:, k, n0:n0 + nw],
                            start=(k == 0), stop=(k == 2 * dc - 1))
                af = []
                for f in range(dc):
                    a_ = apool.tile([128, 512], dt.bfloat16, name="af",
                                    tag="a1")
                    nc.scalar.activation(a_[:, :nw], ps1[f][:, :nw], AF.Relu,
                                         bias=wsb["bfa"][:, f:f + 1])
                    af.append(a_)
                pl = pbig.tile([128, 512], dt.float32, name="psl", tag="mlp")
                for k2 in range(dc):
                    nc.tensor.matmul(
                        pl[:ncls, :nw],
                        lhsT=wsb["wfb"][:, k2, :ncls],
                        rhs=af[k2][:, :nw],
                        start=(k2 == 0), stop=(k2 == dc - 1))
                nc.vector.tensor_scalar(
                    logitsT[:ncls, n0:n0 + nw], pl[:ncls, :nw],
                    wsb["bfb"][:ncls, 0:1], None, ALU.add)

            for tb in range(bt):
                ltp = pps.tile([128, 128], dt.float32, name="ltp",
                               tag="ps")
                nc.tensor.transpose(
                    ltp[:], logitsT[:, tb * 128:(tb + 1) * 128],
                    wsb["ident32"][:])
                mx = mpool.tile([128, 1], dt.float32, name="mx", tag="mx")
                nc.vector.tensor_reduce(mx[:], ltp[:, :ncls],
                                        mybir.AxisListType.X, ALU.max)
                z = mpool.tile([128, ncls], dt.float32, name="z", tag="z")
                nc.vector.tensor_scalar(z[:], ltp[:, :ncls], mx[:, 0:1], None,
                                        ALU.subtract)
                ez = mpool.tile([128, ncls], dt.float32, name="ez", tag="z")
                nc.scalar.activation(ez[:], z[:], AF.Exp)
                sm = mpool.tile([128, 1], dt.float32, name="sm", tag="mx")
                nc.vector.tensor_reduce(sm[:], ez[:], mybir.AxisListType.X,
                                        ALU.add)
                ls = mpool.tile([128, 1], dt.float32, name="ls", tag="mx")
                nc.scalar.activation(ls[:], sm[:], AF.Ln)
                o = mpool.tile([128, ncls], dt.float32, name="o", tag="z")
                nc.vector.tensor_scalar(o[:], z[:], ls[:, 0:1], None,
                                        ALU.subtract)
                nc.sync.dma_start(out_dram[tb * 128:(tb + 1) * 128, :], o[:])

    nc.compile()
    return nc


def build_in_maps(cfg):
    in_maps = []
    for p in range(cfg["ncores"]):
        m = dict(
            xTc=cfg["xTc"][p],
            seg1=cfg["rel"][1]["prep"]["seg"][p],
            idx1=cfg["rel"][1]["prep"]["idx"][p],
            seg2=cfg["rel"][2]["prep"]["seg"][p],
            idx2=cfg["rel"][2]["prep"]["idx"][p],
            dinv2nm1=cfg["rel"][1]["dinv2_nm"][p],
            dinv2nm2=cfg["rel"][2]["dinv2_nm"][p],
            segr1=cfg["ro"][1]["prep"]["seg"][p],
            idxr1=cfg["ro"][1]["prep"]["idx"][p],
            segr2=cfg["ro"][2]["prep"]["seg"][p],
            idxr2=cfg["ro"][2]["prep"]["idx"][p],
            cnts=cfg["cnts"][p],
        )
        m.update({k: v for k, v in cfg["w"].items()})
        in_maps.append(m)
    return in_maps


_CACHE = {}


def kernel(**inputs) -> np.ndarray:
    cfg = host_prep(inputs)
    key = (
        cfg["t_nodes"], cfg["f_in"], cfg["dim"], cfg["ncls"], cfg["n_bins"],
        tuple((cfg["rel"][r]["prep"]["nb_lo"], cfg["rel"][r]["prep"]["nb_hi"])
              for r in (1, 2)),
        tuple(cfg["ro"][i]["prep"]["nb"] for i in (1, 2)),
    )
    if key not in _CACHE:
        _CACHE[key] = build_program(cfg)
    nc = _CACHE[key]

    from concourse.bass_utils import run_bass_kernel_spmd

    in_maps = build_in_maps(cfg)
    res = run_bass_kernel_spmd(nc, in_maps, list(range(cfg["ncores"])))
    outs = [res.results[p]["out"][: cfg["bpc"]] for p in range(cfg["ncores"])]
    return np.ascontiguousarray(np.concatenate(outs, axis=0), np.float32)


# revision 14
# speedup vs baseline: 1.3101x; 1.3101x over previous
"""Trainium2 Bass kernel for nn_Net_50620484551136 (gnn_message_passing).

Network (see problem reference):
  h  = MLP(x)                     # 4652 -> 256 -> 256
  h1 = relu(GCN(h, e1)); h2 = relu(GCN(h, e2))
  h  = MLP([h1, h2])              # 512 -> 256 -> 256
  h1 = relu(GCN(h, e1)); h2 = relu(GCN(h, e2))
  h  = MLP([h1, h2])
  r1 = scatter_mean(h, index_1, N); r2 = scatter_mean(h, index_2, N)
  out = log_softmax(MLP([r1, r2]))

Strategy (8 NeuronCores, SPMD single program):
  - Tuple nodes sharded contiguously across cores (6250/core, padded 6272).
  - Fully fused pipeline: per 4-tile group (512 nodes) of each round we run
    [gathers -> SEG aggregation -> conv -> MLP -> next-round table write] so
    the PE stays continuously busy (p-state!) and no h tensors bounce
    through DRAM.
  - GCN aggregation via PE matmuls against host-built one-hot SEG blocks in
    fp8e4 with perf_mode=DoubleRow (0.5 cycles/row): SEG carries
    dinv[src]*dinv[dst]*SEG_BOOST, gathered table h*G_SCALE in fp8e4, conv
    weights pre-divided by SEG_BOOST*G_SCALE so the natural scale returns.
  - Self-loop (diagonal) term added node-major from resident bf16 gt tiles:
    aggs = agg_psum + gt * (dinv^2 * SEG_BOOST * G_SCALE).
  - Gathers are grouped 4 tiles per dma_gather call (one lo + one hi call
    per group per relation; int16 index split at 32768) with runtime count
    registers trimming trailing pads.
  - Readout (scatter-mean) push-model binned into [5120, 256] with 1/count
    folded into fp8 SEG (x SEG_BOOST), one ReduceScatter(add) per index.
  - Input MLP layer 1 in fp8e4 DoubleRow: x cast to fp8e4, W_i1 * 1024 in
    fp8e4, descaled by 2^-10 in the relu-bias activation.
"""

import numpy as np
import ml_dtypes

BF16 = ml_dtypes.bfloat16
FP8E4 = ml_dtypes.float8_e4m3

# Problem constants (hardcoded per harness contract).
T = 50000
N_BINS = 5000
F_IN = 4652
DIM = 256
N_CLASSES = 5
NCORES = 8
SPLIT = 32768  # int16 gather index limit

SINGLE_PACKET = False
G_SCALE = 16.0       # gathered table stores h * G_SCALE in fp8e4
SEG_BOOST = 64.0     # SEG stores norm * SEG_BOOST in fp8e4
POST_SCALE = 1.0 / (G_SCALE * SEG_BOOST)  # 2^-10, folded into conv weights
FP8_X = True         # input MLP layer 1 in fp8e4 DoubleRow
GROUP = 4            # tiles per gather/conv/MLP group
RO_GROUP = 8         # bin-tiles per readout gather


def _ceil_to(x, m):
    return (x + m - 1) // m * m


def _even(x):
    return x + (x & 1)


def _wrap_idx(v):
    """int16 index vector (len % 16 == 0) -> [128, len/16] wrapped layout."""
    assert len(v) % 16 == 0
    w = v.reshape(-1, 16).T.astype(np.int16)  # [16, len/16]
    return np.tile(w, (8, 1))  # [128, len/16]


def _chunk_weight(w, dtype=BF16, kpad=None):
    """[K, M] -> [128, ceil(K/128), M] (partition = k%128, block = k//128)."""
    k, m = w.shape
    kp = kpad if kpad is not None else _ceil_to(k, 128)
    wp = np.zeros((kp, m), np.float32)
    wp[:k] = w
    return np.ascontiguousarray(
        wp.reshape(kp // 128, 128, m).transpose(1, 0, 2)
    ).astype(dtype)


def _chunk_bias(b):
    """[M] -> [128, ceil(M/128)] f32 (partition = m%128, col = m//128)."""
    m = len(b)
    mp = _ceil_to(m, 128)
    bp = np.zeros(mp, np.float32)
    bp[:m] = b
    return np.ascontiguousarray(bp.reshape(mp // 128, 128).T).astype(np.float32)


def _prep_rel(src, dst, vals, dpc, dpad, ncores, gpos, ngr):
    """Per-core grouped gather idx / SEG / counts for one edge relation.

    dst space sharded dpc per core (padded dpad, nt tiles, groups of GROUP
    tiles).  Source row in the gathered table is gpos[src]; vals[e] is the
    SEG weight (already boosted).  Per group one lo and one hi gather; tile
    j of a group occupies slots [j*nb_lo*128, ...) of the lo region and
    [4*nb_lo*128 + j*nb_hi*128, ...) of the hi region.  Interior pads use
    idx 0 (gathers row 0, annihilated by zero SEG); counts trim the tail.
    """
    nt = dpad // 128
    order = np.argsort(dst, kind="stable")
    src, dst, vals = src[order], dst[order], vals[order]
    core_of = dst // dpc
    gsrc = gpos[src]
    ldst = gpos[dst]

    per_core = []
    nb_lo = 2
    nb_hi = 2
    for p in range(ncores):
        sel = core_of == p
        sp = gsrc[sel]
        vv = vals[sel]
        ld = ldst[sel] - p * dpad
        tiles = []
        for t in range(nt):
            m = (ld // 128) == t
            st = sp[m]
            dd = (ld[m] - t * 128).astype(np.int64)
            va = vv[m]
            lo = st < SPLIT
            ol = np.argsort(st[lo], kind="stable")
            oh = np.argsort(st[~lo], kind="stable")
            tiles.append((st[lo][ol], st[~lo][oh] - SPLIT,
                          dd[lo][ol], dd[~lo][oh], va[lo][ol], va[~lo][oh]))
            nb_lo = max(nb_lo, _even(_ceil_to(max(len(st[lo]), 1), 128) // 128))
            nb_hi = max(nb_hi, _even(_ceil_to(max(len(st[~lo]), 1), 128) // 128))
        per_core.append(tiles)

    nb = nb_lo + nb_hi
    idx_arrs, seg_arrs, cnt_arrs = [], [], []
    for p in range(ncores):
        idx_a = np.full((ngr, 128, GROUP * nb * 8), -1, np.int16)
        seg_a = np.zeros((ngr, 128, GROUP * nb * 128), np.float32)
        cnt_a = np.zeros((nt, 2), np.int32)
        for g in range(ngr):
            gsz = min(GROUP, nt - g * GROUP)
            li_lo = np.full(GROUP * nb_lo * 128, -1, np.int64)
            li_hi = np.full(GROUP * nb_hi * 128, -1, np.int64)
            for j in range(gsz):
                t = g * GROUP + j
                lo_gs, hi_gs, lo_dd, hi_dd, lo_va, hi_va = per_core[p][t]
                o_lo = j * nb_lo * 128
                if len(lo_gs) == 0:
                    li_lo[o_lo] = 0
                li_lo[o_lo:o_lo + len(lo_gs)] = lo_gs
                o_hi = j * nb_hi * 128
                if len(hi_gs) == 0:
                    li_hi[o_hi] = 0
                li_hi[o_hi:o_hi + len(hi_gs)] = hi_gs
                cnt_a[t, 0] = max(len(lo_gs), 1)
                cnt_a[t, 1] = max(len(hi_gs), 1)
                # SEG: lo blocks at (j*nb_lo + b), hi at (GROUP*nb_lo + j*nb_hi + b)
                i = np.arange(len(lo_dd)) + j * nb_lo * 128
                seg_a[g, i % 128,
                      ((i // 128) * 128 + lo_dd)] = lo_va
                i = np.arange(len(hi_dd)) + (GROUP * nb_lo + j * nb_hi) * 128
                seg_a[g, i % 128,
                      ((i // 128) * 128 + hi_dd)] = hi_va
            idx_a[g, :, :GROUP * nb_lo * 8] = _wrap_idx(li_lo.astype(np.int16))
            idx_a[g, :, GROUP * nb_lo * 8:] = _wrap_idx(li_hi.astype(np.int16))
        idx_arrs.append(idx_a)
        seg_arrs.append(np.ascontiguousarray(seg_a.astype(FP8E4)))
        cnt_arrs.append(cnt_a)
    return dict(nb_lo=nb_lo, nb_hi=nb_hi, idx=idx_arrs, seg=seg_arrs,
                cnt=cnt_arrs)


def host_prep(inputs, ncores=NCORES, n_bins=None):
    """Pure-numpy preprocessing: sharding, edge sorting, SEG/idx/count
    construction, weight and x layout."""
    x = np.asarray(inputs["x"], np.float32)
    t_nodes, f_in = x.shape
    dim = np.asarray(inputs["W_i2"]).shape[0]
    ncls = np.asarray(inputs["b_fb"]).shape[0]
    if n_bins is None:
        if t_nodes == T and f_in == F_IN:
            n_bins = N_BINS
        else:
            n_bins = int(np.asarray(inputs["index_1"]).max()) + 1

    assert t_nodes % ncores == 0, (t_nodes, ncores)
    tpc = t_nodes // ncores
    tpad = _ceil_to(tpc, 128)
    nt = tpad // 128
    ngr = _ceil_to(nt, GROUP) // GROUP
    kin = _ceil_to(f_in, 256)  # even number of 128-blocks for DoubleRow
    assert n_bins % ncores == 0, (n_bins, ncores)
    bpc = n_bins // ncores
    bpad = _ceil_to(bpc, 128)
    bt = bpad // 128            # tiles per core's bin shard
    btg = ncores * bt           # global padded bin tiles
    n_rogr = _ceil_to(btg, RO_GROUP) // RO_GROUP

    cfg = dict(
        t_nodes=t_nodes, f_in=f_in, dim=dim, ncls=ncls, n_bins=n_bins,
        ncores=ncores, tpc=tpc, tpad=tpad, nt=nt, ngr=ngr,
        kin=kin, kc=kin // 128,
        bpc=bpc, bpad=bpad, bt=bt, btg=btg, n_rogr=n_rogr,
        g_rows=ncores * tpad,
    )

    # ---- conv relations: drop self-loops via norm folding
    edges = {}
    for r, key in ((1, "edge_index_1"), (2, "edge_index_2")):
        ei = np.asarray(inputs[key]).astype(np.int64)
        s, d = ei[0], ei[1]
        deg = np.bincount(d, minlength=t_nodes).astype(np.float64) + 1.0
        dinv = (1.0 / np.sqrt(deg)).astype(np.float32)
        edges[r] = (s, d, dinv)

    # ---- per-core node permutation balancing per-tile gather-slot counts.
    straddle = SPLIT // tpad  # core whose row range contains SPLIT

    def core_deg4(p, inv_s):
        deg4 = np.zeros((tpc, 4), np.int64)
        for ci, r in enumerate((1, 2)):
            s, d, _ = edges[r]
            sel = (d // tpc) == p
            sl, dl = s[sel], d[sel] - p * tpc
            sc = sl // tpc
            srow = np.where(
                sc == straddle,
                straddle * tpad + inv_s[np.minimum(
                    np.maximum(sl - straddle * tpc, 0), tpc - 1)],
                sc * tpad + (sl % tpc))
            is_lo = srow < SPLIT
            np.add.at(deg4[:, 2 * ci], dl[is_lo], 1)
            np.add.at(deg4[:, 2 * ci + 1], dl[~is_lo], 1)
        return deg4

    ident = np.arange(tpc, dtype=np.int64)
    inv_s = ident  # pass 1: approximate straddle-core positions
    for _ in range(2):
        perm_s = _balance_perm(core_deg4(straddle, inv_s), nt)
        inv_s = np.empty(tpc, np.int64)
        inv_s[perm_s] = ident

    perms = []
    gpos = np.empty(t_nodes, np.int64)
    for p in range(ncores):
        if p == straddle:
            perm = perm_s
        else:
            perm = _balance_perm(core_deg4(p, inv_s), nt)
        perms.append(perm)
        inv = np.empty(tpc, np.int64)
        inv[perm] = ident
        gpos[p * tpc: (p + 1) * tpc] = p * tpad + inv
    cfg["perms"] = perms

    rel = {}
    for r in (1, 2):
        s, d, dinv = edges[r]
        vals = dinv[s] * dinv[d] * SEG_BOOST
        rel[r] = dict(
            prep=_prep_rel(s, d, vals, tpc, tpad, ncores, gpos, ngr),
            dinv=dinv,
        )
    cfg["rel"] = rel

    # ---- readout: push-model over local nodes into global padded bin rows
    ro = {}
    for i, key in ((1, "index_1"), (2, "index_2")):
        idx = np.asarray(inputs[key]).astype(np.int64)
        cnt = np.bincount(idx, minlength=n_bins).astype(np.float64)
        invc = (1.0 / np.maximum(cnt, 1.0)).astype(np.float32) * SEG_BOOST
        grow = (idx // bpc) * bpad + (idx % bpc)  # padded global bin row
        nbro = 2
        percore = []
        for p in range(ncores):
            pm = cfg["perms"][p]
            n_loc = np.arange(tpc, dtype=np.int64)
            g = grow[p * tpc: (p + 1) * tpc][pm]
            v = invc[idx[p * tpc: (p + 1) * tpc][pm]]
            tiles = []
            for tT in range(btg):
                m = (g // 128) == tT
                nn = n_loc[m]
                dd = (g[m] - tT * 128).astype(np.int64)
                vv = v[m]
                o = np.argsort(nn, kind="stable")
                tiles.append((nn[o], dd[o], vv[o]))
                nbro = max(nbro,
                           _even(_ceil_to(max(len(nn), 1), 128) // 128))
            percore.append(tiles)
        idx_arrs, seg_arrs, cnt_arrs = [], [], []
        for p in range(ncores):
            idx_a = np.full((n_rogr, 128, RO_GROUP * nbro * 8), -1, np.int16)
            seg_a = np.zeros((n_rogr, 128, RO_GROUP * nbro * 128), np.float32)
            cnt_a = np.zeros(n_rogr, np.int32)
            for g in range(n_rogr):
                gsz = min(RO_GROUP, btg - g * RO_GROUP)
                li = np.full(RO_GROUP * nbro * 128, -1, np.int64)
                for j in range(gsz):
                    tT = g * RO_GROUP + j
                    nn, dd, vv = percore[p][tT]
                    o = j * nbro * 128
                    li[o:o + nbro * 128] = 0
                    li[o:o + len(nn)] = nn
                    if j == gsz - 1:
                        cnt_a[g] = o + max(len(nn), 1)
                        li[o + max(len(nn), 1):] = -1
                    k = np.arange(len(dd)) + o
                    seg_a[g, k % 128, (k // 128) * 128 + dd] = vv
                idx_a[g] = _wrap_idx(li.astype(np.int16))
            idx_arrs.append(idx_a)
            seg_arrs.append(np.ascontiguousarray(seg_a.astype(FP8E4)))
            cnt_arrs.append(cnt_a)
        ro[i] = dict(prep=dict(nb=nbro, idx=idx_arrs, seg=seg_arrs,
                               cnt=cnt_arrs))
    cfg["ro"] = ro

    # ---- counts tensor per core: [128, CNT_COLS] int32 (replicated rows)
    # layout: rel1 (nt*2: lo,hi), rel2 (nt*2), ro1 (n_rogr), ro2 (n_rogr)
    cnt_cols = 2 * nt * 2 + 2 * n_rogr
    cfg["cnt_cols"] = cnt_cols
    cnts = []
    for p in range(ncores):
        c = np.concatenate([
            rel[1]["prep"]["cnt"][p].reshape(-1),
            rel[2]["prep"]["cnt"][p].reshape(-1),
            ro[1]["prep"]["cnt"][p],
            ro[2]["prep"]["cnt"][p],
        ]).astype(np.int32)
        assert len(c) == cnt_cols
        cnts.append(np.ascontiguousarray(np.tile(c[None, :], (128, 1))))
    cfg["cnts"] = cnts

    # ---- per-core x in chunked layout [NCHUNK, 128, kc*512]
    nch = _ceil_to(tpad, 512) // 512
    cfg["nch"] = nch
    kc = kin // 128
    xdt = FP8E4 if FP8_X else BF16
    xTc = []
    for p in range(ncores):
        xs = np.zeros((kin, nch * 512), np.float32)
        xs[:f_in, :tpc] = x[p * tpc: (p + 1) * tpc][cfg["perms"][p]].T
        a = np.ascontiguousarray(
            xs.reshape(kc, 128, nch, 512).transpose(2, 1, 0, 3)
            .reshape(nch, 128, kc * 512)
        ).astype(xdt)
        xTc.append(a)
    cfg["xTc"] = xTc

    # ---- dinv^2 node-major [128, nt] f32 per relation per core
    # (partition = node slot within tile; value dinv^2 * SEG_BOOST * G_SCALE
    #  so that gt(bf16, natural h) * this == 1024 * h * dinv^2)
    for r in (1, 2):
        dn = []
        dinv2 = rel[r]["dinv"] ** 2 * (SEG_BOOST * G_SCALE)
        for p in range(ncores):
            vp = np.zeros(tpad, np.float32)
            vp[:tpc] = dinv2[p * tpc: (p + 1) * tpc][cfg["perms"][p]]
            dn.append(np.ascontiguousarray(
                vp.reshape(nt, 128).T.astype(np.float32)))
        rel[r]["dinv2_nm"] = dn

    # ---- weights
    w = {}
    if FP8_X:
        w["wi1"] = _chunk_weight(
            np.asarray(inputs["W_i1"], np.float32) * (G_SCALE * SEG_BOOST),
            FP8E4, kpad=kin)
    else:
        w["wi1"] = _chunk_weight(np.asarray(inputs["W_i1"], np.float32),
                                 kpad=kin)
    w["wi2"] = _chunk_weight(np.asarray(inputs["W_i2"], np.float32))
    for nm, src in (("wc11", "Wc11"), ("wc12", "Wc12"),
                    ("wc21", "Wc21"), ("wc22", "Wc22")):
        w[nm] = _chunk_weight(np.asarray(inputs[src], np.float32) * POST_SCALE)
    for nm, src in (("wm1a", "W_m1a"), ("wm1b", "W_m1b"),
                    ("wm2a", "W_m2a"), ("wm2b", "W_m2b"),
                    ("wfa", "W_fa"), ("wfb", "W_fb")):
        w[nm] = _chunk_weight(np.asarray(inputs[src], np.float32))
    for nm, src in (("bi1", "b_i1"), ("bi2", "b_i2"),
                    ("bc11", "bc11"), ("bc12", "bc12"),
                    ("bc21", "bc21"), ("bc22", "bc22"),
                    ("bm1a", "b_m1a"), ("bm1b", "b_m1b"),
                    ("bm2a", "b_m2a"), ("bm2b", "b_m2b"),
                    ("bfa", "b_fa"), ("bfb", "b_fb")):
        w[nm] = _chunk_bias(np.asarray(inputs[src], np.float32))
    w["ident16"] = np.eye(128, dtype=BF16)
    w["ident32"] = np.eye(128, dtype=np.float32)
    cfg["w"] = w
    return cfg


def _balance_perm(deg4, nt, cap=128):
    """Greedy assignment of nodes to tiles balancing 4 degree components."""
    n_nodes = deg4.shape[0]
    order = np.argsort(-deg4.sum(1), kind="stable")
    loads = np.zeros((nt, 4))
    counts = np.zeros(nt, np.int64)
    capv = np.full(nt, cap, np.int64)
    capv[-1] = n_nodes - (nt - 1) * cap
    wscale = 1.0 / np.maximum(deg4.mean(0), 1e-9)
    tiles = [[] for _ in range(nt)]
    for n in order:
        avail = np.nonzero(counts < capv)[0]
        after = ((loads[avail] + deg4[n]) * wscale).max(1)
        j = avail[np.argmin(after + 1e-6 * loads[avail].sum(1))]
        tiles[j].append(n)
        loads[j] += deg4[n]
        counts[j] += 1
    perm = np.empty(n_nodes, np.int64)
    for t in range(nt):
        sl = np.sort(np.array(tiles[t], np.int64))
        perm[t * cap: t * cap + len(sl)] = sl
    return perm


def _nchunks(total, step):
    out = []
    o = 0
    while o < total:
        out.append((o, min(step, total - o)))
        o += step
    return out


def build_program(cfg):
    """Build the SPMD bass program (one program, 8 cores)."""
    import concourse.bass as bass
    import concourse.mybir as mybir
    import concourse.tile as tile
    from concourse import bacc

    dt = mybir.dt
    AF = mybir.ActivationFunctionType
    ALU = mybir.AluOpType
    DR = mybir.MatmulPerfMode.DoubleRow

    nt, tpad, kc = cfg["nt"], cfg["tpad"], cfg["kc"]
    ngr = cfg["ngr"]
    bt, bpad, btg = cfg["bt"], cfg["bpad"], cfg["btg"]
    n_rogr = cfg["n_rogr"]
    dim, ncls = cfg["dim"], cfg["ncls"]
    dc = dim // 128
    g_rows = cfg["g_rows"]
    ncores = cfg["ncores"]
    nch = cfg["nch"]
    rel, ro = cfg["rel"], cfg["ro"]
    rg = [list(range(ncores))]

    nb_r = {r: rel[r]["prep"]["nb_lo"] + rel[r]["prep"]["nb_hi"]
            for r in (1, 2)}
    nbro = {i: ro[i]["prep"]["nb"] for i in (1, 2)}
    # shared ed/seg/idx pool block capacity
    nb_max = max(max(GROUP * nb_r[r] for r in (1, 2)),
                 max(RO_GROUP * nbro[i] for i in (1, 2)))

    GDT = dt.float8e4
    XDT = dt.float8e4 if FP8_X else dt.bfloat16

    nc = bacc.Bacc("TRN2", target_bir_lowering=False, debug=False,
                   num_devices=ncores, num_swdge_queues=4)
    qstate = [0]

    def next_q():
        q = qstate[0]
        qstate[0] = (q + 1) % 4
        return q

    # ---------------- I/O declarations ----------------
    xTc = nc.dram_tensor("xTc", [nch, 128, kc * 512], XDT,
                         kind="ExternalInput")
    seg_in, idx_in, dinv2_in = {}, {}, {}
    for r in (1, 2):
        nb = nb_r[r]
        seg_in[r] = nc.dram_tensor(f"seg{r}", [ngr, 128, GROUP * nb * 128],
                                   dt.float8e4, kind="ExternalInput")
        idx_in[r] = nc.dram_tensor(f"idx{r}", [ngr, 128, GROUP * nb * 8],
                                   dt.int16, kind="ExternalInput")
        dinv2_in[r] = nc.dram_tensor(f"dinv2nm{r}", [128, nt], dt.float32,
                                     kind="ExternalInput")
    segr_in, idxr_in = {}, {}
    for i in (1, 2):
        nb = nbro[i]
        segr_in[i] = nc.dram_tensor(f"segr{i}",
                                    [n_rogr, 128, RO_GROUP * nb * 128],
                                    dt.float8e4, kind="ExternalInput")
        idxr_in[i] = nc.dram_tensor(f"idxr{i}",
                                    [n_rogr, 128, RO_GROUP * nb * 8],
                                    dt.int16, kind="ExternalInput")
    cnts_in = nc.dram_tensor("cnts", [128, cfg["cnt_cols"]], dt.int32,
                             kind="ExternalInput")

    wnames_w = dict(
        wi2=[128, dc, dim],
        wc11=[128, dc, dim], wc12=[128, dc, dim],
        wc21=[128, dc, dim], wc22=[128, dc, dim],
        wm1a=[128, 2 * dc, dim], wm1b=[128, dc, dim],
        wm2a=[128, 2 * dc, dim], wm2b=[128, dc, dim],
        wfa=[128, 2 * dc, dim], wfb=[128, dc, ncls],
        ident16=[128, 128],
    )
    wnames_f32 = dict(
        bi1=[128, dc], bi2=[128, dc],
        bc11=[128, dc], bc12=[128, dc], bc21=[128, dc], bc22=[128, dc],
        bm1a=[128, dc], bm1b=[128, dc], bm2a=[128, dc], bm2b=[128, dc],
        bfa=[128, dc], bfb=[128, 1],
        ident32=[128, 128],
    )
    win = {}
    win["wi1"] = nc.dram_tensor("wi1", [128, kc, dim], XDT,
                                kind="ExternalInput")
    for nm, shp in wnames_w.items():
        win[nm] = nc.dram_tensor(nm, shp, dt.bfloat16, kind="ExternalInput")
    for nm, shp in wnames_f32.items():
        win[nm] = nc.dram_tensor(nm, shp, dt.float32, kind="ExternalInput")

    out_dram = nc.dram_tensor("out", [bpad, ncls], dt.float32,
                              kind="ExternalOutput")

    # counts column offsets
    def cnt_col_rel(r, t, hi):
        return (r - 1) * nt * 2 + t * 2 + (1 if hi else 0)

    def cnt_col_ro(i, g):
        return 2 * nt * 2 + (i - 1) * n_rogr + g

    from contextlib import ExitStack

    with tile.TileContext(nc) as tc, ExitStack() as es:
        if True:
            wpool = es.enter_context(tc.tile_pool(name="wpool", bufs=1))
            xpool = es.enter_context(tc.tile_pool(name="xpool", bufs=2))
            hstgp = es.enter_context(tc.tile_pool(name="hstg", bufs=2))
            houtp = es.enter_context(tc.tile_pool(name="hout", bufs=2))
            aggtp = es.enter_context(tc.tile_pool(name="aggtp", bufs=2))
            apool = es.enter_context(tc.tile_pool(name="apool", bufs=3))
            edpool = es.enter_context(tc.tile_pool(name="edpool", bufs=4))
            segpool = es.enter_context(tc.tile_pool(name="segpool", bufs=4))
            idxpool = es.enter_context(tc.tile_pool(name="idxpool", bufs=5))
            gtpool = es.enter_context(tc.tile_pool(name="gtpool", bufs=1))
            g8pool = es.enter_context(tc.tile_pool(name="g8pool", bufs=2))
            mpool = es.enter_context(tc.tile_pool(name="mpool", bufs=3))
            pbig = es.enter_context(
                tc.tile_pool(name="pbig", bufs=3, space="PSUM"))
            pps = es.enter_context(
                tc.tile_pool(name="pps", bufs=3, space="PSUM"))
            pcnv = es.enter_context(
                tc.tile_pool(name="pcnv", bufs=2, space="PSUM"))
            dpool = es.enter_context(
                tc.tile_pool(name="dpool", bufs=1, space="DRAM"))
            # ---- resident weights + counts
            wsb = {}
            wt = wpool.tile([128, kc, dim], XDT, name="sb_wi1", tag="w_wi1")
            nc.sync.dma_start(wt[:], win["wi1"][:])
            wsb["wi1"] = wt
            for nm in list(wnames_w) + list(wnames_f32):
                shp = wnames_w.get(nm) or wnames_f32[nm]
                dtyp = dt.bfloat16 if nm in wnames_w else dt.float32
                wt = wpool.tile(shp, dtyp, name=f"sb_{nm}", tag=f"w_{nm}")
                nc.sync.dma_start(wt[:], win[nm][:])
                wsb[nm] = wt
            dinv2_sb = {}
            for r in (1, 2):
                dv = wpool.tile([128, nt], dt.float32, name=f"sb_dinv2{r}",
                                tag=f"w_dinv2{r}")
                nc.sync.dma_start(dv[:], dinv2_in[r][:])
                dinv2_sb[r] = dv
            cnts_sb = wpool.tile([128, cfg["cnt_cols"]], dt.int32,
                                 name="sb_cnts", tag="w_cnts")
            nc.sync.dma_start(cnts_sb[:], cnts_in[:])
            cregs = [nc.gpsimd.alloc_register(f"gather_cnt{q}")
                     for q in range(4)]

            # zero the gather destination pool once so skipped (padded) slots
            # always hold finite stale values (SEG zeros annihilate them)
            for b in range(4):
                edt = edpool.tile([128, nb_max, dim], GDT, name="ed", tag="ed")
                nc.vector.memset(edt[:], 0.0)

            # resident bf16 node-major h tiles (diag term source)
            gt_bf = [gtpool.tile([128, dim], dt.bfloat16, name=f"gtb{t}",
                                 tag=f"gtb{t}") for t in range(nt)]

            g_loc, g_full = {}, {}
            for rnd in (1, 2):
                g_loc[rnd] = dpool.tile([tpad, dim], GDT, name=f"g_loc{rnd}",
                                        tag=f"g_loc{rnd}")
                g_full[rnd] = dpool.tile([g_rows, dim], GDT,
                                         name=f"g_full{rnd}",
                                         tag=f"g_full{rnd}",
                                         addr_space="Shared")
            hf_loc = dpool.tile([tpad, dim], GDT, name="hf_loc", tag="hf_loc")

            def table_write(hstg, g, gsz, dst):
                """Transpose feature-major hstg [128, dc, 512] chunk into
                node-major bf16 gt tiles + fp8 rows of dst (x G_SCALE)."""
                for j in range(gsz):
                    t = g * GROUP + j
                    g8 = g8pool.tile([128, dim], GDT, name="g8", tag="g8")
                    for f in range(dc):
                        tp = pps.tile([128, 128], dt.bfloat16, name="tw",
                                       tag="ps")
                        nc.tensor.transpose(
                            tp[:], hstg[:, f, j * 128:(j + 1) * 128],
                            wsb["ident16"][:])
                        nc.vector.tensor_copy(
                            gt_bf[t][:, f * 128:(f + 1) * 128], tp[:])
                        nc.scalar.activation(
                            g8[:, f * 128:(f + 1) * 128], tp[:], AF.Copy,
                            scale=G_SCALE)
                    nc.sync.dma_start(dst[t * 128:(t + 1) * 128, :], g8[:])

            # =========== Phase 1: input MLP  h0 = relu(x@Wi1+bi1)@Wi2+bi2
            kgs = _nchunks(kc, 8)  # k-groups of 8 k-blocks (all even)
            for g in range(ngr):
                gsz = min(GROUP, nt - g * GROUP)
                n0 = g * 512
                nw = gsz * 128
                ps1 = [pbig.tile([128, 512], dt.float32, name=f"ps1_{f}",
                                 tag="mlp") for f in range(dc)]
                for (k0, kw) in kgs:
                    xg = xpool.tile([128, 8, 512], XDT, name="xg", tag="xg")
                    nc.sync.dma_start(
                        xg[:, :kw, :],
                        xTc[g, :, k0 * 512: (k0 + kw) * 512])
                    if FP8_X:
                        for ki in range(0, kw, 2):
                            k = k0 + ki
                            for f in range(dc):
                                for h0 in range(0, nw, 256):
                                    hw_ = min(256, nw - h0)
                                    nc.tensor.matmul(
                                        ps1[f][:, h0:h0 + hw_],
                                        lhsT=wsb["wi1"][:, k:k + 2,
                                                        f * 128:(f + 1) * 128],
                                        rhs=xg[:, ki:ki + 2, h0:h0 + hw_],
                                        start=(k == 0), stop=(k == kc - 2),
                                        perf_mode=DR)
                    else:
                        for ki in range(kw):
                            k = k0 + ki
                            for f in range(dc):
                                nc.tensor.matmul(
                                    ps1[f][:, :nw],
                                    lhsT=wsb["wi1"][:, k,
                                                    f * 128:(f + 1) * 128],
                                    rhs=xg[:, ki, :nw],
                                    start=(k == 0), stop=(k == kc - 1))
                a1 = []
                for f in range(dc):
                    a_ = apool.tile([128, 512], dt.bfloat16, name=f"a1_{f}",
                                    tag="a1")
                    nc.scalar.activation(
                        a_[:, :nw], ps1[f][:, :nw], AF.Relu,
                        bias=wsb["bi1"][:, f:f + 1],
                        scale=(POST_SCALE if FP8_X else 1.0))
                    a1.append(a_)
                hstg = hstgp.tile([128, dc, 512], dt.bfloat16, name="h0s",
                                  tag="hstg")
                for f2 in range(dc):
                    p2 = pbig.tile([128, 512], dt.float32, name="ps2",
                                   tag="mlp")
                    for k2 in range(dc):
                        nc.tensor.matmul(
                            p2[:, :nw],
                            lhsT=wsb["wi2"][:, k2, f2 * 128:(f2 + 1) * 128],
                            rhs=a1[k2][:, :nw],
                            start=(k2 == 0), stop=(k2 == dc - 1))
                    nc.vector.tensor_scalar(
                        hstg[:, f2, :nw], p2[:, :nw],
                        wsb["bi2"][:, f2:f2 + 1], None, ALU.add)
                table_write(hstg, g, gsz, g_loc[1][:])
            nc.gpsimd.collective_compute(
                "AllGather", ALU.bypass, replica_groups=rg,
                ins=[g_loc[1][:]], outs=[g_full[1][:]])

            # =========== Phase 2: two fused GCN+MLP rounds
            for rnd in (1, 2):
                wma = wsb[f"wm{rnd}a"]
                wmb = wsb[f"wm{rnd}b"]
                bma = wsb[f"bm{rnd}a"]
                bmb = wsb[f"bm{rnd}b"]
                for g in range(ngr):
                    gsz = min(GROUP, nt - g * GROUP)
                    nw = gsz * 128
                    eds, segs, idxs = {}, {}, {}
                    for r in (1, 2):
                        nb = nb_r[r]
                        idxt = idxpool.tile([128, nb_max * 8], dt.int16,
                                            name="idxt", tag="idx")
                        nc.sync.dma_start(idxt[:, :GROUP * nb * 8],
                                          idx_in[r][g])
                        idxs[r] = idxt
                        segt = segpool.tile([128, nb_max, 128], dt.float8e4,
                                            name="segt", tag="seg")
                        nc.scalar.dma_start(segt[:, :GROUP * nb, :],
                                            seg_in[r][g])
                        segs[r] = segt
                    for r in (1, 2):
                        pr = rel[r]["prep"]
                        nb_lo, nb_hi = pr["nb_lo"], pr["nb_hi"]
                        nb = nb_lo + nb_hi
                        idxt = idxs[r]
                        ed = edpool.tile([128, nb_max, dim], GDT,
                                         name="ed", tag="ed")
                        for j in range(gsz):
                            t = g * GROUP + j
                            o = j * nb_lo
                            q = next_q()
                            nc.gpsimd.reg_load(
                                cregs[q],
                                cnts_sb[0:1, cnt_col_rel(r, t, False):
                                        cnt_col_rel(r, t, False) + 1])
                            nc.gpsimd.dma_gather(
                                ed[:, o:o + nb_lo, :], g_full[rnd][:],
                                idxt[:, o * 8:(o + nb_lo) * 8],
                                nb_lo * 128, cregs[q], dim,
                                single_packet=SINGLE_PACKET, queue_num=q)
                            oh = GROUP * nb_lo + j * nb_hi
                            q = next_q()
                            nc.gpsimd.reg_load(
                                cregs[q],
                                cnts_sb[0:1, cnt_col_rel(r, t, True):
                                        cnt_col_rel(r, t, True) + 1])
                            nc.gpsimd.dma_gather(
                                ed[:, oh:oh + nb_hi, :],
                                g_full[rnd][SPLIT:g_rows, :],
                                idxt[:, oh * 8:(oh + nb_hi) * 8],
                                nb_hi * 128, cregs[q], dim,
                                single_packet=SINGLE_PACKET, queue_num=q)
                        eds[r] = ed

                    aggT = {r: aggtp.tile([128, dc, 512], dt.bfloat16,
                                          name=f"aggT{r}", tag=f"aggT{r}")
                            for r in (1, 2)}
                    for j in range(gsz):
                        t = g * GROUP + j
                        for r in (1, 2):
                            pr = rel[r]["prep"]
                            nb_lo, nb_hi = pr["nb_lo"], pr["nb_hi"]
                            ed, segt = eds[r], segs[r]
                            agg = pps.tile([128, dim], dt.float32,
                                            name="agg", tag="ps")
                            npair = (nb_lo + nb_hi) // 2
                            bi = 0
                            for bp in range(nb_lo // 2):
                                off = j * nb_lo + 2 * bp
                                nc.tensor.matmul(
                                    agg[:],
                                    lhsT=segt[:, off:off + 2, :],
                                    rhs=ed[:, off:off + 2, :],
                                    start=(bi == 0), stop=(bi == npair - 1),
                                    perf_mode=DR)
                                bi += 1
                            for bp in range(nb_hi // 2):
                                off = GROUP * nb_lo + j * nb_hi + 2 * bp
                                nc.tensor.matmul(
                                    agg[:],
                                    lhsT=segt[:, off:off + 2, :],
                                    rhs=ed[:, off:off + 2, :],
                                    start=(bi == 0), stop=(bi == npair - 1),
                                    perf_mode=DR)
                                bi += 1
                            # diagonal term (node-major): gt * dinv2*1024
                            tmpd = mpool.tile([128, dim], dt.bfloat16,
                                              name="tmpd", tag="tmpd")
                            nc.vector.tensor_scalar(
                                tmpd[:], gt_bf[t][:],
                                dinv2_sb[r][:, t:t + 1], None, ALU.mult)
                            aggs = mpool.tile([128, dim], dt.bfloat16,
                                              name="aggs", tag="aggs")
                            nc.vector.tensor_tensor(
                                aggs[:], agg[:], tmpd[:], ALU.add)
                            for f in range(dc):
                                tp = pps.tile([128, 128], dt.bfloat16,
                                               name="tpc", tag="ps")
                                nc.tensor.transpose(
                                    tp[:], aggs[:, f * 128:(f + 1) * 128],
                                    wsb["ident16"][:])
                                nc.vector.tensor_copy(
                                    aggT[r][:, f, j * 128:(j + 1) * 128],
                                    tp[:])
                    # conv (batched over the group)
                    hout = {}
                    for r in (1, 2):
                        wc = wsb[f"wc{rnd}{r}"]
                        bc = wsb[f"bc{rnd}{r}"]
                        ho = houtp.tile([128, dc, 512], dt.bfloat16,
                                        name=f"ho{r}", tag=f"ho{r}")
                        for f2 in range(dc):
                            pc = pcnv.tile([128, 512], dt.float32,
                                           name="pc", tag="cnv")
                            for k in range(dc):
                                nc.tensor.matmul(
                                    pc[:, :nw],
                                    lhsT=wc[:, k, f2 * 128:(f2 + 1) * 128],
                                    rhs=aggT[r][:, k, :nw],
                                    start=(k == 0), stop=(k == dc - 1))
                            nc.vector.tensor_scalar(
                                ho[:, f2, :nw], pc[:, :nw],
                                bc[:, f2:f2 + 1], 0.0, ALU.add, ALU.max)
                        hout[r] = ho
                    # MLP on concat(h1, h2) for this group's nodes
                    ps1 = [pbig.tile([128, 512], dt.float32, name="psm1",
                                     tag="mlp") for f in range(dc)]
                    for k in range(2 * dc):
                        rhs_t = hout[1] if k < dc else hout[2]
                        for f in range(dc):
                            nc.tensor.matmul(
                                ps1[f][:, :nw],
                                lhsT=wma[:, k, f * 128:(f + 1) * 128],
                                rhs=rhs_t[:, k % dc, :nw],
                                start=(k == 0), stop=(k == 2 * dc - 1))
                    am = []
                    for f in range(dc):
                        a_ = apool.tile([128, 512], dt.bfloat16, name="am",
                                        tag="a1")
                        nc.scalar.activation(a_[:, :nw], ps1[f][:, :nw],
                                             AF.Relu, bias=bma[:, f:f + 1])
                        am.append(a_)
                    hstg = hstgp.tile([128, dc, 512], dt.bfloat16,
                                      name="hms", tag="hstg")
                    for f2 in range(dc):
                        p2 = pbig.tile([128, 512], dt.float32, name="psm2",
                                       tag="mlp")
                        for k2 in range(dc):
                            nc.tensor.matmul(
                                p2[:, :nw],
                                lhsT=wmb[:, k2, f2 * 128:(f2 + 1) * 128],
                                rhs=am[k2][:, :nw],
                                start=(k2 == 0), stop=(k2 == dc - 1))
                        nc.vector.tensor_scalar(
                            hstg[:, f2, :nw], p2[:, :nw],
                            bmb[:, f2:f2 + 1], None, ALU.add)
                    # table write for the next stage
                    dst = g_loc[2][:] if rnd == 1 else hf_loc[:]
                    table_write(hstg, g, gsz, dst)
                if rnd == 1:
                    nc.gpsimd.collective_compute(
                        "AllGather", ALU.bypass, replica_groups=rg,
                        ins=[g_loc[2][:]], outs=[g_full[2][:]])

            # =========== Phase 3: readout (push + ReduceScatter)
            parts = {}
            for i in (1, 2):
                pr = ro[i]["prep"]
                nb = pr["nb"]
                part = dpool.tile([ncores * bpad, dim], dt.bfloat16,
                                  name=f"part{i}", tag=f"part{i}")
                for g in range(n_rogr):
                    gsz = min(RO_GROUP, btg - g * RO_GROUP)
                    idxt = idxpool.tile([128, nb_max * 8], dt.int16,
                                        name="idxtr", tag="idx")
                    nc.sync.dma_start(idxt[:, :RO_GROUP * nb * 8],
                                      idxr_in[i][g])
                    segt = segpool.tile([128, nb_max, 128], dt.float8e4,
                                        name="segtr", tag="seg")
                    nc.scalar.dma_start(segt[:, :RO_GROUP * nb, :],
                                        segr_in[i][g])
                    ed = edpool.tile([128, nb_max, dim], GDT,
                                     name="edr", tag="ed")
                    q = next_q()
                    nc.gpsimd.reg_load(
                        cregs[q], cnts_sb[0:1, cnt_col_ro(i, g):
                                          cnt_col_ro(i, g) + 1])
                    nc.gpsimd.dma_gather(
                        ed[:, 0:RO_GROUP * nb, :], hf_loc[:],
                        idxt[:, 0:RO_GROUP * nb * 8],
                        RO_GROUP * nb * 128, cregs[q], dim,
                        single_packet=SINGLE_PACKET, queue_num=q)
                    for j in range(gsz):
                        tT = g * RO_GROUP + j
                        agg = pps.tile([128, dim], dt.float32, name="aggr",
                                        tag="ps")
                        npair = nb // 2
                        for bp in range(npair):
                            off = j * nb + 2 * bp
                            nc.tensor.matmul(
                                agg[:],
                                lhsT=segt[:, off:off + 2, :],
                                rhs=ed[:, off:off + 2, :],
                                start=(bp == 0), stop=(bp == npair - 1),
                                perf_mode=DR)
                        aggs = mpool.tile([128, dim], dt.bfloat16,
                                          name="aggsr", tag="aggs")
                        nc.scalar.activation(aggs[:], agg[:], AF.Copy,
                                             scale=POST_SCALE)
                        nc.sync.dma_start(
                            part[tT * 128:(tT + 1) * 128, :], aggs[:])
                parts[i] = part
            rsh = {}
            for i in (1, 2):
                rs = dpool.tile([bpad, dim], dt.bfloat16, name=f"rsh{i}",
                                tag=f"rsh{i}")
                nc.gpsimd.collective_compute(
                    "ReduceScatter", ALU.add, replica_groups=rg,
                    ins=[parts[i][:]], outs=[rs[:]])
                rsh[i] = rs

            # transpose RS shards to feature-major rcat [128, 2*dc, bpad]
            rcat = wpool.tile([128, 2 * dc, bpad], dt.bfloat16, name="rcat",
                              tag="rcat")
            for i in (1, 2):
                for tb in range(bt):
                    rt = mpool.tile([128, dim], dt.bfloat16, name="rt",
                                    tag="rt")
                    nc.sync.dma_start(rt[:],
                                      rsh[i][tb * 128:(tb + 1) * 128, :])
                    for f in range(dc):
                        tp = pps.tile([128, 128], dt.bfloat16, name="tpr",
                                       tag="ps")
                        nc.tensor.transpose(
                            tp[:], rt[:, f * 128:(f + 1) * 128],
                            wsb["ident16"][:])
                        nc.vector.tensor_copy(
                            rcat[:, (i - 1) * dc + f,
                                 tb * 128:(tb + 1) * 128], tp[:])

            # ---- final MLP + log_softmax
            logitsT = wpool.tile([128, bpad], dt.float32, name="logitsT",
                                 tag="logitsT")
            for (n0, nw) in _nchunks(bpad, 512):
                ps1 = [pbig.tile([128, 512], dt.float32, name="psf1",
                                 tag="mlp") for f in range(dc)]
                for k in range(2 * dc):
                    for f in range(dc):
                        nc.tensor.matmul(
                            ps1[f][:, :nw],
                            lhsT=wsb["wfa"][:, k, f * 128:(f + 1) * 128],
                            rhs=rcat[:, k, n0:n0 + nw],
                            start=(k == 0), stop=(k == 2 * dc - 1))
                af = []
                for f in range(dc):
                    a_ = apool.tile([128, 512], dt.bfloat16, name="af",
                                    tag="a1")
                    nc.scalar.activation(a_[:, :nw], ps1[f][:, :nw], AF.Relu,
                                         bias=wsb["bfa"][:, f:f + 1])
                    af.append(a_)
                pl = pbig.tile([128, 512], dt.float32, name="psl", tag="mlp")
                for k2 in range(dc):
                    nc.tensor.matmul(
                        pl[:ncls, :nw],
                        lhsT=wsb["wfb"][:, k2, :ncls],
                        rhs=af[k2][:, :nw],
                        start=(k2 == 0), stop=(k2 == dc - 1))
                nc.vector.tensor_scalar(
                    logitsT[:ncls, n0:n0 + nw], pl[:ncls, :nw],
                    wsb["bfb"][:ncls, 0:1], None, ALU.add)

            for tb in range(bt):
                ltp = pps.tile([128, 128], dt.float32, name="ltp",
                               tag="ps")
                nc.tensor.transpose(
                    ltp[:], logitsT[:, tb * 128:(tb + 1) * 128],
                    wsb["ident32"][:])
                mx = mpool.tile([128, 1], dt.float32, name="mx", tag="mx")
                nc.vector.tensor_reduce(mx[:], ltp[:, :ncls],
                                        mybir.AxisListType.X, ALU.max)
                z = mpool.tile([128, ncls], dt.float32, name="z", tag="z")
                nc.vector.tensor_scalar(z[:], ltp[:, :ncls], mx[:, 0:1], None,
                                        ALU.subtract)
                ez = mpool.tile([128, ncls], dt.float32, name="ez", tag="z")
                nc.scalar.activation(ez[:], z[:], AF.Exp)
                sm = mpool.tile([128, 1], dt.float32, name="sm", tag="mx")
                nc.vector.tensor_reduce(sm[:], ez[:], mybir.AxisListType.X,
                                        ALU.add)
                ls = mpool.tile([128, 1], dt.float32, name="ls", tag="mx")
                nc.scalar.activation(ls[:], sm[:], AF.Ln)
                o = mpool.tile([128, ncls], dt.float32, name="o", tag="z")
                nc.vector.tensor_scalar(o[:], z[:], ls[:, 0:1], None,
                                        ALU.subtract)
                nc.sync.dma_start(out_dram[tb * 128:(tb + 1) * 128, :], o[:])

    nc.compile()
    return nc


def build_in_maps(cfg):
    in_maps = []
    for p in range(cfg["ncores"]):
        m = dict(
            xTc=cfg["xTc"][p],
            seg1=cfg["rel"][1]["prep"]["seg"][p],
            idx1=cfg["rel"][1]["prep"]["idx"][p],
            seg2=cfg["rel"][2]["prep"]["seg"][p],
            idx2=cfg["rel"][2]["prep"]["idx"][p],
            dinv2nm1=cfg["rel"][1]["dinv2_nm"][p],
            dinv2nm2=cfg["rel"][2]["dinv2_nm"][p],
            segr1=cfg["ro"][1]["prep"]["seg"][p],
            idxr1=cfg["ro"][1]["prep"]["idx"][p],
            segr2=cfg["ro"][2]["prep"]["seg"][p],
            idxr2=cfg["ro"][2]["prep"]["idx"][p],
            cnts=cfg["cnts"][p],
        )
        m.update({k: v for k, v in cfg["w"].items()})
        in_maps.append(m)
    return in_maps


_CACHE = {}


def kernel(**inputs) -> np.ndarray:
    cfg = host_prep(inputs)
    key = (
        cfg["t_nodes"], cfg["f_in"], cfg["dim"], cfg["ncls"], cfg["n_bins"],
        tuple((cfg["rel"][r]["prep"]["nb_lo"], cfg["rel"][r]["prep"]["nb_hi"])
              for r in (1, 2)),
        tuple(cfg["ro"][i]["prep"]["nb"] for i in (1, 2)),
    )
    if key not in _CACHE:
        _CACHE[key] = build_program(cfg)
    nc = _CACHE[key]

    from concourse.bass_utils import run_bass_kernel_spmd

    in_maps = build_in_maps(cfg)
    res = run_bass_kernel_spmd(nc, in_maps, list(range(cfg["ncores"])))
    outs = [res.results[p]["out"][: cfg["bpc"]] for p in range(cfg["ncores"])]
    return np.ascontiguousarray(np.concatenate(outs, axis=0), np.float32)


# revision 16
# speedup vs baseline: 1.3343x; 1.0184x over previous
"""Trainium2 Bass kernel for nn_Net_50620484551136 (gnn_message_passing).

Network (see problem reference):
  h  = MLP(x)                     # 4652 -> 256 -> 256
  h1 = relu(GCN(h, e1)); h2 = relu(GCN(h, e2))
  h  = MLP([h1, h2])              # 512 -> 256 -> 256
  h1 = relu(GCN(h, e1)); h2 = relu(GCN(h, e2))
  h  = MLP([h1, h2])
  r1 = scatter_mean(h, index_1, N); r2 = scatter_mean(h, index_2, N)
  out = log_softmax(MLP([r1, r2]))

Strategy (8 NeuronCores, SPMD single program):
  - Tuple nodes sharded contiguously across cores (6250/core, padded 6272).
  - Fully fused pipeline: per 4-tile group (512 nodes) of each round we run
    [gathers -> SEG aggregation -> conv -> MLP -> next-round table write] so
    the PE stays continuously busy (p-state!) and no h tensors bounce
    through DRAM.
  - GCN aggregation via PE matmuls against host-built one-hot SEG blocks in
    fp8e4 with perf_mode=DoubleRow (0.5 cycles/row): SEG carries
    dinv[src]*dinv[dst]*SEG_BOOST, gathered table h*G_SCALE in fp8e4, conv
    weights pre-divided by SEG_BOOST*G_SCALE so the natural scale returns.
  - Self-loop (diagonal) term added node-major from resident bf16 gt tiles:
    aggs = agg_psum + gt * (dinv^2 * SEG_BOOST * G_SCALE).
  - Gathers are grouped 4 tiles per dma_gather call (one lo + one hi call
    per group per relation; int16 index split at 32768) with runtime count
    registers trimming trailing pads.
  - Readout (scatter-mean) push-model binned into [5120, 256] with 1/count
    folded into fp8 SEG (x SEG_BOOST), one ReduceScatter(add) per index.
  - Input MLP layer 1 in fp8e4 DoubleRow: x cast to fp8e4, W_i1 * 1024 in
    fp8e4, descaled by 2^-10 in the relu-bias activation.
"""

import numpy as np
import ml_dtypes

BF16 = ml_dtypes.bfloat16
FP8E4 = ml_dtypes.float8_e4m3

# Problem constants (hardcoded per harness contract).
T = 50000
N_BINS = 5000
F_IN = 4652
DIM = 256
N_CLASSES = 5
NCORES = 8
SPLIT = 32768  # int16 gather index limit

SINGLE_PACKET = False
G_SCALE = 16.0       # gathered table stores h * G_SCALE in fp8e4
SEG_BOOST = 64.0     # SEG stores norm * SEG_BOOST in fp8e4
POST_SCALE = 1.0 / (G_SCALE * SEG_BOOST)  # 2^-10, folded into conv weights
FP8_X = True         # input MLP layer 1 in fp8e4 DoubleRow
GROUP = 4            # tiles per gather/conv/MLP group
RO_GROUP = 8         # bin-tiles per readout gather


def _ceil_to(x, m):
    return (x + m - 1) // m * m


def _even(x):
    return x + (x & 1)


def _wrap_idx(v):
    """int16 index vector (len % 16 == 0) -> [128, len/16] wrapped layout."""
    assert len(v) % 16 == 0
    w = v.reshape(-1, 16).T.astype(np.int16)  # [16, len/16]
    return np.tile(w, (8, 1))  # [128, len/16]


def _chunk_weight(w, dtype=BF16, kpad=None):
    """[K, M] -> [128, ceil(K/128), M] (partition = k%128, block = k//128)."""
    k, m = w.shape
    kp = kpad if kpad is not None else _ceil_to(k, 128)
    wp = np.zeros((kp, m), np.float32)
    wp[:k] = w
    return np.ascontiguousarray(
        wp.reshape(kp // 128, 128, m).transpose(1, 0, 2)
    ).astype(dtype)


def _chunk_bias(b):
    """[M] -> [128, ceil(M/128)] f32 (partition = m%128, col = m//128)."""
    m = len(b)
    mp = _ceil_to(m, 128)
    bp = np.zeros(mp, np.float32)
    bp[:m] = b
    return np.ascontiguousarray(bp.reshape(mp // 128, 128).T).astype(np.float32)


def _prep_rel(src, dst, vals, dpc, dpad, ncores, gpos, ngr):
    """Per-core grouped gather idx / SEG / counts for one edge relation.

    dst space sharded dpc per core (padded dpad, nt tiles, groups of GROUP
    tiles).  Source row in the gathered table is gpos[src]; vals[e] is the
    SEG weight (already boosted).  Per group one lo and one hi gather; tile
    j of a group occupies slots [j*nb_lo*128, ...) of the lo region and
    [4*nb_lo*128 + j*nb_hi*128, ...) of the hi region.  Interior pads use
    idx 0 (gathers row 0, annihilated by zero SEG); counts trim the tail.
    """
    nt = dpad // 128
    order = np.argsort(dst, kind="stable")
    src, dst, vals = src[order], dst[order], vals[order]
    core_of = dst // dpc
    gsrc = gpos[src]
    ldst = gpos[dst]

    per_core = []
    nb_lo = 2
    nb_hi = 2
    for p in range(ncores):
        sel = core_of == p
        sp = gsrc[sel]
        vv = vals[sel]
        ld = ldst[sel] - p * dpad
        tiles = []
        for t in range(nt):
            m = (ld // 128) == t
            st = sp[m]
            dd = (ld[m] - t * 128).astype(np.int64)
            va = vv[m]
            lo = st < SPLIT
            ol = np.argsort(st[lo], kind="stable")
            oh = np.argsort(st[~lo], kind="stable")
            tiles.append((st[lo][ol], st[~lo][oh] - SPLIT,
                          dd[lo][ol], dd[~lo][oh], va[lo][ol], va[~lo][oh]))
            nb_lo = max(nb_lo, _even(_ceil_to(max(len(st[lo]), 1), 128) // 128))
            nb_hi = max(nb_hi, _even(_ceil_to(max(len(st[~lo]), 1), 128) // 128))
        per_core.append(tiles)

    nb = nb_lo + nb_hi
    idx_arrs, seg_arrs, cnt_arrs = [], [], []
    for p in range(ncores):
        idx_a = np.full((ngr, 128, GROUP * nb * 8), -1, np.int16)
        seg_a = np.zeros((ngr, 128, GROUP * nb * 128), np.float32)
        cnt_a = np.zeros((nt, 2), np.int32)
        for g in range(ngr):
            gsz = min(GROUP, nt - g * GROUP)
            li_lo = np.full(GROUP * nb_lo * 128, -1, np.int64)
            li_hi = np.full(GROUP * nb_hi * 128, -1, np.int64)
            for j in range(gsz):
                t = g * GROUP + j
                lo_gs, hi_gs, lo_dd, hi_dd, lo_va, hi_va = per_core[p][t]
                o_lo = j * nb_lo * 128
                if len(lo_gs) == 0:
                    li_lo[o_lo] = 0
                li_lo[o_lo:o_lo + len(lo_gs)] = lo_gs
                o_hi = j * nb_hi * 128
                if len(hi_gs) == 0:
                    li_hi[o_hi] = 0
                li_hi[o_hi:o_hi + len(hi_gs)] = hi_gs
                cnt_a[t, 0] = max(len(lo_gs), 1)
                cnt_a[t, 1] = max(len(hi_gs), 1)
                # SEG: lo blocks at (j*nb_lo + b), hi at (GROUP*nb_lo + j*nb_hi + b)
                i = np.arange(len(lo_dd)) + j * nb_lo * 128
                seg_a[g, i % 128,
                      ((i // 128) * 128 + lo_dd)] = lo_va
                i = np.arange(len(hi_dd)) + (GROUP * nb_lo + j * nb_hi) * 128
                seg_a[g, i % 128,
                      ((i // 128) * 128 + hi_dd)] = hi_va
            idx_a[g, :, :GROUP * nb_lo * 8] = _wrap_idx(li_lo.astype(np.int16))
            idx_a[g, :, GROUP * nb_lo * 8:] = _wrap_idx(li_hi.astype(np.int16))
        idx_arrs.append(idx_a)
        seg_arrs.append(np.ascontiguousarray(seg_a.astype(FP8E4)))
        cnt_arrs.append(cnt_a)
    return dict(nb_lo=nb_lo, nb_hi=nb_hi, idx=idx_arrs, seg=seg_arrs,
                cnt=cnt_arrs)


def host_prep(inputs, ncores=NCORES, n_bins=None):
    """Pure-numpy preprocessing: sharding, edge sorting, SEG/idx/count
    construction, weight and x layout."""
    x = np.asarray(inputs["x"], np.float32)
    t_nodes, f_in = x.shape
    dim = np.asarray(inputs["W_i2"]).shape[0]
    ncls = np.asarray(inputs["b_fb"]).shape[0]
    if n_bins is None:
        if t_nodes == T and f_in == F_IN:
            n_bins = N_BINS
        else:
            n_bins = int(np.asarray(inputs["index_1"]).max()) + 1

    assert t_nodes % ncores == 0, (t_nodes, ncores)
    tpc = t_nodes // ncores
    tpad = _ceil_to(tpc, 128)
    nt = tpad // 128
    ngr = _ceil_to(nt, GROUP) // GROUP
    kin = _ceil_to(f_in, 256)  # even number of 128-blocks for DoubleRow
    assert n_bins % ncores == 0, (n_bins, ncores)
    bpc = n_bins // ncores
    bpad = _ceil_to(bpc, 128)
    bt = bpad // 128            # tiles per core's bin shard
    btg = ncores * bt           # global padded bin tiles
    n_rogr = _ceil_to(btg, RO_GROUP) // RO_GROUP

    cfg = dict(
        t_nodes=t_nodes, f_in=f_in, dim=dim, ncls=ncls, n_bins=n_bins,
        ncores=ncores, tpc=tpc, tpad=tpad, nt=nt, ngr=ngr,
        kin=kin, kc=kin // 128,
        bpc=bpc, bpad=bpad, bt=bt, btg=btg, n_rogr=n_rogr,
        g_rows=ncores * tpad,
    )

    # ---- conv relations: drop self-loops via norm folding
    edges = {}
    for r, key in ((1, "edge_index_1"), (2, "edge_index_2")):
        ei = np.asarray(inputs[key]).astype(np.int64)
        s, d = ei[0], ei[1]
        deg = np.bincount(d, minlength=t_nodes).astype(np.float64) + 1.0
        dinv = (1.0 / np.sqrt(deg)).astype(np.float32)
        edges[r] = (s, d, dinv)

    # ---- per-core node permutation balancing per-tile gather-slot counts.
    straddle = SPLIT // tpad  # core whose row range contains SPLIT

    def core_deg4(p, inv_s):
        deg4 = np.zeros((tpc, 4), np.int64)
        for ci, r in enumerate((1, 2)):
            s, d, _ = edges[r]
            sel = (d // tpc) == p
            sl, dl = s[sel], d[sel] - p * tpc
            sc = sl // tpc
            srow = np.where(
                sc == straddle,
                straddle * tpad + inv_s[np.minimum(
                    np.maximum(sl - straddle * tpc, 0), tpc - 1)],
                sc * tpad + (sl % tpc))
            is_lo = srow < SPLIT
            np.add.at(deg4[:, 2 * ci], dl[is_lo], 1)
            np.add.at(deg4[:, 2 * ci + 1], dl[~is_lo], 1)
        return deg4

    ident = np.arange(tpc, dtype=np.int64)
    inv_s = ident  # pass 1: approximate straddle-core positions
    for _ in range(2):
        perm_s = _balance_perm(core_deg4(straddle, inv_s), nt)
        inv_s = np.empty(tpc, np.int64)
        inv_s[perm_s] = ident

    perms = []
    gpos = np.empty(t_nodes, np.int64)
    for p in range(ncores):
        if p == straddle:
            perm = perm_s
        else:
            perm = _balance_perm(core_deg4(p, inv_s), nt)
        perms.append(perm)
        inv = np.empty(tpc, np.int64)
        inv[perm] = ident
        gpos[p * tpc: (p + 1) * tpc] = p * tpad + inv
    cfg["perms"] = perms

    rel = {}
    for r in (1, 2):
        s, d, dinv = edges[r]
        vals = dinv[s] * dinv[d] * SEG_BOOST
        rel[r] = dict(
            prep=_prep_rel(s, d, vals, tpc, tpad, ncores, gpos, ngr),
            dinv=dinv,
        )
    cfg["rel"] = rel

    # ---- readout: push-model over local nodes into global padded bin rows
    ro = {}
    for i, key in ((1, "index_1"), (2, "index_2")):
        idx = np.asarray(inputs[key]).astype(np.int64)
        cnt = np.bincount(idx, minlength=n_bins).astype(np.float64)
        invc = (1.0 / np.maximum(cnt, 1.0)).astype(np.float32) * SEG_BOOST
        grow = (idx // bpc) * bpad + (idx % bpc)  # padded global bin row
        nbro = 2
        percore = []
        for p in range(ncores):
            pm = cfg["perms"][p]
            n_loc = np.arange(tpc, dtype=np.int64)
            g = grow[p * tpc: (p + 1) * tpc][pm]
            v = invc[idx[p * tpc: (p + 1) * tpc][pm]]
            tiles = []
            for tT in range(btg):
                m = (g // 128) == tT
                nn = n_loc[m]
                dd = (g[m] - tT * 128).astype(np.int64)
                vv = v[m]
                o = np.argsort(nn, kind="stable")
                tiles.append((nn[o], dd[o], vv[o]))
                nbro = max(nbro,
                           _even(_ceil_to(max(len(nn), 1), 128) // 128))
            percore.append(tiles)
        idx_arrs, seg_arrs, cnt_arrs = [], [], []
        for p in range(ncores):
            idx_a = np.full((n_rogr, 128, RO_GROUP * nbro * 8), -1, np.int16)
            seg_a = np.zeros((n_rogr, 128, RO_GROUP * nbro * 128), np.float32)
            cnt_a = np.zeros(n_rogr, np.int32)
            for g in range(n_rogr):
                gsz = min(RO_GROUP, btg - g * RO_GROUP)
                li = np.full(RO_GROUP * nbro * 128, -1, np.int64)
                for j in range(gsz):
                    tT = g * RO_GROUP + j
                    nn, dd, vv = percore[p][tT]
                    o = j * nbro * 128
                    li[o:o + nbro * 128] = 0
                    li[o:o + len(nn)] = nn
                    if j == gsz - 1:
                        cnt_a[g] = o + max(len(nn), 1)
                        li[o + max(len(nn), 1):] = -1
                    k = np.arange(len(dd)) + o
                    seg_a[g, k % 128, (k // 128) * 128 + dd] = vv
                idx_a[g] = _wrap_idx(li.astype(np.int16))
            idx_arrs.append(idx_a)
            seg_arrs.append(np.ascontiguousarray(seg_a.astype(FP8E4)))
            cnt_arrs.append(cnt_a)
        ro[i] = dict(prep=dict(nb=nbro, idx=idx_arrs, seg=seg_arrs,
                               cnt=cnt_arrs))
    cfg["ro"] = ro

    # ---- counts tensor per core: [128, CNT_COLS] int32 (replicated rows)
    # layout: rel1 (nt*2: lo,hi), rel2 (nt*2), ro1 (n_rogr), ro2 (n_rogr)
    cnt_cols = 2 * nt * 2 + 2 * n_rogr
    cfg["cnt_cols"] = cnt_cols
    cnts = []
    for p in range(ncores):
        c = np.concatenate([
            rel[1]["prep"]["cnt"][p].reshape(-1),
            rel[2]["prep"]["cnt"][p].reshape(-1),
            ro[1]["prep"]["cnt"][p],
            ro[2]["prep"]["cnt"][p],
        ]).astype(np.int32)
        assert len(c) == cnt_cols
        cnts.append(np.ascontiguousarray(np.tile(c[None, :], (128, 1))))
    cfg["cnts"] = cnts

    # ---- per-core x in chunked layout [NCHUNK, 128, kc*512]
    nch = _ceil_to(tpad, 512) // 512
    cfg["nch"] = nch
    kc = kin // 128
    xdt = FP8E4 if FP8_X else BF16
    xTc = []
    for p in range(ncores):
        xs = np.zeros((kin, nch * 512), np.float32)
        xs[:f_in, :tpc] = x[p * tpc: (p + 1) * tpc][cfg["perms"][p]].T
        a = np.ascontiguousarray(
            xs.reshape(kc, 128, nch, 512).transpose(2, 1, 0, 3)
            .reshape(nch, 128, kc * 512)
        ).astype(xdt)
        xTc.append(a)
    cfg["xTc"] = xTc

    # ---- dinv^2 node-major [128, nt] f32 per relation per core
    # (partition = node slot within tile; value dinv^2 * SEG_BOOST * G_SCALE
    #  so that gt(bf16, natural h) * this == 1024 * h * dinv^2)
    for r in (1, 2):
        dn = []
        dinv2 = rel[r]["dinv"] ** 2 * (SEG_BOOST * G_SCALE)
        for p in range(ncores):
            vp = np.zeros(tpad, np.float32)
            vp[:tpc] = dinv2[p * tpc: (p + 1) * tpc][cfg["perms"][p]]
            dn.append(np.ascontiguousarray(
                vp.reshape(nt, 128).T.astype(np.float32)))
        rel[r]["dinv2_nm"] = dn

    # ---- weights
    w = {}
    if FP8_X:
        w["wi1"] = _chunk_weight(
            np.asarray(inputs["W_i1"], np.float32) * (G_SCALE * SEG_BOOST),
            FP8E4, kpad=kin)
    else:
        w["wi1"] = _chunk_weight(np.asarray(inputs["W_i1"], np.float32),
                                 kpad=kin)
    w["wi2"] = _chunk_weight(np.asarray(inputs["W_i2"], np.float32))
    for nm, src in (("wc11", "Wc11"), ("wc12", "Wc12"),
                    ("wc21", "Wc21"), ("wc22", "Wc22")):
        w[nm] = _chunk_weight(np.asarray(inputs[src], np.float32) * POST_SCALE)
    for nm, src in (("wm1a", "W_m1a"), ("wm1b", "W_m1b"),
                    ("wm2a", "W_m2a"), ("wm2b", "W_m2b"),
                    ("wfa", "W_fa"), ("wfb", "W_fb")):
        w[nm] = _chunk_weight(np.asarray(inputs[src], np.float32))
    for nm, src in (("bi1", "b_i1"), ("bi2", "b_i2"),
                    ("bc11", "bc11"), ("bc12", "bc12"),
                    ("bc21", "bc21"), ("bc22", "bc22"),
                    ("bm1a", "b_m1a"), ("bm1b", "b_m1b"),
                    ("bm2a", "b_m2a"), ("bm2b", "b_m2b"),
                    ("bfa", "b_fa"), ("bfb", "b_fb")):
        w[nm] = _chunk_bias(np.asarray(inputs[src], np.float32))
    w["ident16"] = np.eye(128, dtype=BF16)
    w["ident32"] = np.eye(128, dtype=np.float32)
    cfg["w"] = w
    return cfg


def _balance_perm(deg4, nt, cap=128):
    """Greedy assignment of nodes to tiles balancing 4 degree components."""
    n_nodes = deg4.shape[0]
    order = np.argsort(-deg4.sum(1), kind="stable")
    loads = np.zeros((nt, 4))
    counts = np.zeros(nt, np.int64)
    capv = np.full(nt, cap, np.int64)
    capv[-1] = n_nodes - (nt - 1) * cap
    wscale = 1.0 / np.maximum(deg4.mean(0), 1e-9)
    tiles = [[] for _ in range(nt)]
    for n in order:
        avail = np.nonzero(counts < capv)[0]
        after = ((loads[avail] + deg4[n]) * wscale).max(1)
        j = avail[np.argmin(after + 1e-6 * loads[avail].sum(1))]
        tiles[j].append(n)
        loads[j] += deg4[n]
        counts[j] += 1
    perm = np.empty(n_nodes, np.int64)
    for t in range(nt):
        sl = np.sort(np.array(tiles[t], np.int64))
        perm[t * cap: t * cap + len(sl)] = sl
    return perm


def _nchunks(total, step):
    out = []
    o = 0
    while o < total:
        out.append((o, min(step, total - o)))
        o += step
    return out


def build_program(cfg):
    """Build the SPMD bass program (one program, 8 cores)."""
    import concourse.bass as bass
    import concourse.mybir as mybir
    import concourse.tile as tile
    from concourse import bacc

    dt = mybir.dt
    AF = mybir.ActivationFunctionType
    ALU = mybir.AluOpType
    DR = mybir.MatmulPerfMode.DoubleRow

    nt, tpad, kc = cfg["nt"], cfg["tpad"], cfg["kc"]
    ngr = cfg["ngr"]
    bt, bpad, btg = cfg["bt"], cfg["bpad"], cfg["btg"]
    n_rogr = cfg["n_rogr"]
    dim, ncls = cfg["dim"], cfg["ncls"]
    dc = dim // 128
    g_rows = cfg["g_rows"]
    ncores = cfg["ncores"]
    nch = cfg["nch"]
    rel, ro = cfg["rel"], cfg["ro"]
    rg = [list(range(ncores))]

    nb_r = {r: rel[r]["prep"]["nb_lo"] + rel[r]["prep"]["nb_hi"]
            for r in (1, 2)}
    nbro = {i: ro[i]["prep"]["nb"] for i in (1, 2)}
    # shared ed/seg/idx pool block capacity
    nb_max = max(max(GROUP * nb_r[r] for r in (1, 2)),
                 max(RO_GROUP * nbro[i] for i in (1, 2)))

    GDT = dt.float8e4
    XDT = dt.float8e4 if FP8_X else dt.bfloat16

    nc = bacc.Bacc("TRN2", target_bir_lowering=False, debug=False,
                   num_devices=ncores, num_swdge_queues=4)
    qstate = [0]

    def next_q():
        q = qstate[0]
        qstate[0] = (q + 1) % 4
        return q

    # ---------------- I/O declarations ----------------
    xTc = nc.dram_tensor("xTc", [nch, 128, kc * 512], XDT,
                         kind="ExternalInput")
    seg_in, idx_in, dinv2_in = {}, {}, {}
    for r in (1, 2):
        nb = nb_r[r]
        seg_in[r] = nc.dram_tensor(f"seg{r}", [ngr, 128, GROUP * nb * 128],
                                   dt.float8e4, kind="ExternalInput")
        idx_in[r] = nc.dram_tensor(f"idx{r}", [ngr, 128, GROUP * nb * 8],
                                   dt.int16, kind="ExternalInput")
        dinv2_in[r] = nc.dram_tensor(f"dinv2nm{r}", [128, nt], dt.float32,
                                     kind="ExternalInput")
    segr_in, idxr_in = {}, {}
    for i in (1, 2):
        nb = nbro[i]
        segr_in[i] = nc.dram_tensor(f"segr{i}",
                                    [n_rogr, 128, RO_GROUP * nb * 128],
                                    dt.float8e4, kind="ExternalInput")
        idxr_in[i] = nc.dram_tensor(f"idxr{i}",
                                    [n_rogr, 128, RO_GROUP * nb * 8],
                                    dt.int16, kind="ExternalInput")
    cnts_in = nc.dram_tensor("cnts", [128, cfg["cnt_cols"]], dt.int32,
                             kind="ExternalInput")

    wnames_w = dict(
        wi2=[128, dc, dim],
        wc11=[128, dc, dim], wc12=[128, dc, dim],
        wc21=[128, dc, dim], wc22=[128, dc, dim],
        wm1a=[128, 2 * dc, dim], wm1b=[128, dc, dim],
        wm2a=[128, 2 * dc, dim], wm2b=[128, dc, dim],
        wfa=[128, 2 * dc, dim], wfb=[128, dc, ncls],
        ident16=[128, 128],
    )
    wnames_f32 = dict(
        bi1=[128, dc], bi2=[128, dc],
        bc11=[128, dc], bc12=[128, dc], bc21=[128, dc], bc22=[128, dc],
        bm1a=[128, dc], bm1b=[128, dc], bm2a=[128, dc], bm2b=[128, dc],
        bfa=[128, dc], bfb=[128, 1],
        ident32=[128, 128],
    )
    win = {}
    win["wi1"] = nc.dram_tensor("wi1", [128, kc, dim], XDT,
                                kind="ExternalInput")
    for nm, shp in wnames_w.items():
        win[nm] = nc.dram_tensor(nm, shp, dt.bfloat16, kind="ExternalInput")
    for nm, shp in wnames_f32.items():
        win[nm] = nc.dram_tensor(nm, shp, dt.float32, kind="ExternalInput")

    out_dram = nc.dram_tensor("out", [bpad, ncls], dt.float32,
                              kind="ExternalOutput")

    # counts column offsets
    def cnt_col_rel(r, t, hi):
        return (r - 1) * nt * 2 + t * 2 + (1 if hi else 0)

    def cnt_col_ro(i, g):
        return 2 * nt * 2 + (i - 1) * n_rogr + g

    from contextlib import ExitStack

    with tile.TileContext(nc) as tc, ExitStack() as es:
        if True:
            wpool = es.enter_context(tc.tile_pool(name="wpool", bufs=1))
            xpool = es.enter_context(tc.tile_pool(name="xpool", bufs=2))
            hstgp = es.enter_context(tc.tile_pool(name="hstg", bufs=2))
            houtp = es.enter_context(tc.tile_pool(name="hout", bufs=2))
            aggtp = es.enter_context(tc.tile_pool(name="aggtp", bufs=2))
            apool = es.enter_context(tc.tile_pool(name="apool", bufs=3))
            edpool = es.enter_context(tc.tile_pool(name="edpool", bufs=4))
            segpool = es.enter_context(tc.tile_pool(name="segpool", bufs=4))
            idxpool = es.enter_context(tc.tile_pool(name="idxpool", bufs=5))
            gtpool = es.enter_context(tc.tile_pool(name="gtpool", bufs=1))
            g8pool = es.enter_context(tc.tile_pool(name="g8pool", bufs=2))
            mpool = es.enter_context(tc.tile_pool(name="mpool", bufs=3))
            pbig = es.enter_context(
                tc.tile_pool(name="pbig", bufs=4, space="PSUM"))
            paggp = es.enter_context(
                tc.tile_pool(name="paggp", bufs=2, space="PSUM"))
            pcnv = es.enter_context(
                tc.tile_pool(name="pcnv", bufs=2, space="PSUM"))
            dpool = es.enter_context(
                tc.tile_pool(name="dpool", bufs=1, space="DRAM"))
            # ---- resident weights + counts
            wsb = {}
            wt = wpool.tile([128, kc, dim], XDT, name="sb_wi1", tag="w_wi1")
            nc.sync.dma_start(wt[:], win["wi1"][:])
            wsb["wi1"] = wt
            for nm in list(wnames_w) + list(wnames_f32):
                shp = wnames_w.get(nm) or wnames_f32[nm]
                dtyp = dt.bfloat16 if nm in wnames_w else dt.float32
                wt = wpool.tile(shp, dtyp, name=f"sb_{nm}", tag=f"w_{nm}")
                nc.sync.dma_start(wt[:], win[nm][:])
                wsb[nm] = wt
            dinv2_sb = {}
            for r in (1, 2):
                dv = wpool.tile([128, nt], dt.float32, name=f"sb_dinv2{r}",
                                tag=f"w_dinv2{r}")
                nc.sync.dma_start(dv[:], dinv2_in[r][:])
                dinv2_sb[r] = dv
            cnts_sb = wpool.tile([128, cfg["cnt_cols"]], dt.int32,
                                 name="sb_cnts", tag="w_cnts")
            nc.sync.dma_start(cnts_sb[:], cnts_in[:])
            cregs = [nc.gpsimd.alloc_register(f"gather_cnt{q}")
                     for q in range(4)]

            # zero the gather destination pool once so skipped (padded) slots
            # always hold finite stale values (SEG zeros annihilate them)
            for b in range(4):
                edt = edpool.tile([128, nb_max, dim], GDT, name="ed", tag="ed")
                nc.vector.memset(edt[:], 0.0)

            # resident bf16 node-major h tiles (diag term source)
            gt_bf = [gtpool.tile([128, dim], dt.bfloat16, name=f"gtb{t}",
                                 tag=f"gtb{t}") for t in range(nt)]

            g_loc, g_full = {}, {}
            for rnd in (1, 2):
                g_loc[rnd] = dpool.tile([tpad, dim], GDT, name=f"g_loc{rnd}",
                                        tag=f"g_loc{rnd}")
                g_full[rnd] = dpool.tile([g_rows, dim], GDT,
                                         name=f"g_full{rnd}",
                                         tag=f"g_full{rnd}",
                                         addr_space="Shared")
            hf_loc = dpool.tile([tpad, dim], GDT, name="hf_loc", tag="hf_loc")

            def table_write(hstg, g, gsz, dst):
                """Transpose feature-major hstg [128, dc, 512] chunk into
                node-major bf16 gt tiles + fp8 rows of dst (x G_SCALE)."""
                for j in range(gsz):
                    t = g * GROUP + j
                    g8 = g8pool.tile([128, dim], GDT, name="g8", tag="g8")
                    for f in range(dc):
                        tp = pcnv.tile([128, 128], dt.bfloat16, name="tw",
                                       tag="cnv")
                        nc.tensor.transpose(
                            tp[:], hstg[:, f, j * 128:(j + 1) * 128],
                            wsb["ident16"][:])
                        nc.vector.tensor_copy(
                            gt_bf[t][:, f * 128:(f + 1) * 128], tp[:])
                        nc.scalar.activation(
                            g8[:, f * 128:(f + 1) * 128], tp[:], AF.Copy,
                            scale=G_SCALE)
                    nc.sync.dma_start(dst[t * 128:(t + 1) * 128, :], g8[:])

            # =========== Phase 1: input MLP  h0 = relu(x@Wi1+bi1)@Wi2+bi2
            kgs = _nchunks(kc, 8)  # k-groups of 8 k-blocks (all even)
            for g in range(ngr):
                gsz = min(GROUP, nt - g * GROUP)
                n0 = g * 512
                nw = gsz * 128
                ps1 = [pbig.tile([128, 512], dt.float32, name=f"ps1_{f}",
                                 tag="mlp") for f in range(dc)]
                for (k0, kw) in kgs:
                    xg = xpool.tile([128, 8, 512], XDT, name="xg", tag="xg")
                    nc.sync.dma_start(
                        xg[:, :kw, :],
                        xTc[g, :, k0 * 512: (k0 + kw) * 512])
                    if FP8_X:
                        for ki in range(0, kw, 2):
                            k = k0 + ki
                            for f in range(dc):
                                for h0 in range(0, nw, 256):
                                    hw_ = min(256, nw - h0)
                                    nc.tensor.matmul(
                                        ps1[f][:, h0:h0 + hw_],
                                        lhsT=wsb["wi1"][:, k:k + 2,
                                                        f * 128:(f + 1) * 128],
                                        rhs=xg[:, ki:ki + 2, h0:h0 + hw_],
                                        start=(k == 0), stop=(k == kc - 2),
                                        perf_mode=DR)
                    else:
                        for ki in range(kw):
                            k = k0 + ki
                            for f in range(dc):
                                nc.tensor.matmul(
                                    ps1[f][:, :nw],
                                    lhsT=wsb["wi1"][:, k,
                                                    f * 128:(f + 1) * 128],
                                    rhs=xg[:, ki, :nw],
                                    start=(k == 0), stop=(k == kc - 1))
                a1 = []
                for f in range(dc):
                    a_ = apool.tile([128, 512], dt.bfloat16, name=f"a1_{f}",
                                    tag="a1")
                    nc.scalar.activation(
                        a_[:, :nw], ps1[f][:, :nw], AF.Relu,
                        bias=wsb["bi1"][:, f:f + 1],
                        scale=(POST_SCALE if FP8_X else 1.0))
                    a1.append(a_)
                hstg = hstgp.tile([128, dc, 512], dt.bfloat16, name="h0s",
                                  tag="hstg")
                for f2 in range(dc):
                    p2 = pbig.tile([128, 512], dt.float32, name="ps2",
                                   tag="mlp")
                    for k2 in range(dc):
                        nc.tensor.matmul(
                            p2[:, :nw],
                            lhsT=wsb["wi2"][:, k2, f2 * 128:(f2 + 1) * 128],
                            rhs=a1[k2][:, :nw],
                            start=(k2 == 0), stop=(k2 == dc - 1))
                    nc.vector.tensor_scalar(
                        hstg[:, f2, :nw], p2[:, :nw],
                        wsb["bi2"][:, f2:f2 + 1], None, ALU.add)
                table_write(hstg, g, gsz, g_loc[1][:])
            nc.gpsimd.collective_compute(
                "AllGather", ALU.bypass, replica_groups=rg,
                ins=[g_loc[1][:]], outs=[g_full[1][:]])

            # =========== Phase 2: two fused GCN+MLP rounds
            for rnd in (1, 2):
                wma = wsb[f"wm{rnd}a"]
                wmb = wsb[f"wm{rnd}b"]
                bma = wsb[f"bm{rnd}a"]
                bmb = wsb[f"bm{rnd}b"]
                for g in range(ngr):
                    gsz = min(GROUP, nt - g * GROUP)
                    nw = gsz * 128
                    eds, segs, idxs = {}, {}, {}
                    for r in (1, 2):
                        nb = nb_r[r]
                        idxt = idxpool.tile([128, nb_max * 8], dt.int16,
                                            name="idxt", tag="idx")
                        nc.sync.dma_start(idxt[:, :GROUP * nb * 8],
                                          idx_in[r][g])
                        idxs[r] = idxt
                        segt = segpool.tile([128, nb_max, 128], dt.float8e4,
                                            name="segt", tag="seg")
                        nc.scalar.dma_start(segt[:, :GROUP * nb, :],
                                            seg_in[r][g])
                        segs[r] = segt
                    for r in (1, 2):
                        pr = rel[r]["prep"]
                        nb_lo, nb_hi = pr["nb_lo"], pr["nb_hi"]
                        nb = nb_lo + nb_hi
                        idxt = idxs[r]
                        ed = edpool.tile([128, nb_max, dim], GDT,
                                         name="ed", tag="ed")
                        for j in range(gsz):
                            t = g * GROUP + j
                            o = j * nb_lo
                            q = next_q()
                            nc.gpsimd.reg_load(
                                cregs[q],
                                cnts_sb[0:1, cnt_col_rel(r, t, False):
                                        cnt_col_rel(r, t, False) + 1])
                            nc.gpsimd.dma_gather(
                                ed[:, o:o + nb_lo, :], g_full[rnd][:],
                                idxt[:, o * 8:(o + nb_lo) * 8],
                                nb_lo * 128, cregs[q], dim,
                                single_packet=SINGLE_PACKET, queue_num=q)
                            oh = GROUP * nb_lo + j * nb_hi
                            q = next_q()
                            nc.gpsimd.reg_load(
                                cregs[q],
                                cnts_sb[0:1, cnt_col_rel(r, t, True):
                                        cnt_col_rel(r, t, True) + 1])
                            nc.gpsimd.dma_gather(
                                ed[:, oh:oh + nb_hi, :],
                                g_full[rnd][SPLIT:g_rows, :],
                                idxt[:, oh * 8:(oh + nb_hi) * 8],
                                nb_hi * 128, cregs[q], dim,
                                single_packet=SINGLE_PACKET, queue_num=q)
                        eds[r] = ed

                    aggT = {r: aggtp.tile([128, dc, 512], dt.bfloat16,
                                          name=f"aggT{r}", tag=f"aggT{r}")
                            for r in (1, 2)}
                    for j in range(gsz):
                        t = g * GROUP + j
                        # diagonal terms first (independent of the gathers)
                        tmpds = {}
                        for r in (1, 2):
                            tmpd = mpool.tile([128, dim], dt.bfloat16,
                                              name="tmpd", tag="tmpd")
                            nc.vector.tensor_scalar(
                                tmpd[:], gt_bf[t][:],
                                dinv2_sb[r][:, t:t + 1], None, ALU.mult)
                            tmpds[r] = tmpd
                        aggs_t = {}
                        for r in (1, 2):
                            pr = rel[r]["prep"]
                            nb_lo, nb_hi = pr["nb_lo"], pr["nb_hi"]
                            ed, segt = eds[r], segs[r]
                            agg = paggp.tile([128, dim], dt.float32,
                                             name="agg", tag="agg")
                            npair = (nb_lo + nb_hi) // 2
                            bi = 0
                            for bp in range(nb_lo // 2):
                                off = j * nb_lo + 2 * bp
                                nc.tensor.matmul(
                                    agg[:],
                                    lhsT=segt[:, off:off + 2, :],
                                    rhs=ed[:, off:off + 2, :],
                                    start=(bi == 0), stop=(bi == npair - 1),
                                    perf_mode=DR)
                                bi += 1
                            for bp in range(nb_hi // 2):
                                off = GROUP * nb_lo + j * nb_hi + 2 * bp
                                nc.tensor.matmul(
                                    agg[:],
                                    lhsT=segt[:, off:off + 2, :],
                                    rhs=ed[:, off:off + 2, :],
                                    start=(bi == 0), stop=(bi == npair - 1),
                                    perf_mode=DR)
                                bi += 1
                            aggs = mpool.tile([128, dim], dt.bfloat16,
                                              name="aggs", tag="aggs")
                            nc.vector.tensor_tensor(
                                aggs[:], agg[:], tmpds[r][:], ALU.add)
                            aggs_t[r] = aggs
                        for r in (1, 2):
                            for f in range(dc):
                                tp = pcnv.tile([128, 128], dt.bfloat16,
                                               name="tpc", tag="cnv")
                                nc.tensor.transpose(
                                    tp[:],
                                    aggs_t[r][:, f * 128:(f + 1) * 128],
                                    wsb["ident16"][:])
                                nc.vector.tensor_copy(
                                    aggT[r][:, f, j * 128:(j + 1) * 128],
                                    tp[:])
                    # conv (batched over the group)
                    hout = {}
                    for r in (1, 2):
                        wc = wsb[f"wc{rnd}{r}"]
                        bc = wsb[f"bc{rnd}{r}"]
                        ho = houtp.tile([128, dc, 512], dt.bfloat16,
                                        name=f"ho{r}", tag=f"ho{r}")
                        for f2 in range(dc):
                            pc = pcnv.tile([128, 512], dt.float32,
                                           name="pc", tag="cnv")
                            for k in range(dc):
                                nc.tensor.matmul(
                                    pc[:, :nw],
                                    lhsT=wc[:, k, f2 * 128:(f2 + 1) * 128],
                                    rhs=aggT[r][:, k, :nw],
                                    start=(k == 0), stop=(k == dc - 1))
                            nc.vector.tensor_scalar(
                                ho[:, f2, :nw], pc[:, :nw],
                                bc[:, f2:f2 + 1], 0.0, ALU.add, ALU.max)
                        hout[r] = ho
                    # MLP on concat(h1, h2) for this group's nodes
                    ps1 = [pbig.tile([128, 512], dt.float32, name="psm1",
                                     tag="mlp") for f in range(dc)]
                    for k in range(2 * dc):
                        rhs_t = hout[1] if k < dc else hout[2]
                        for f in range(dc):
                            nc.tensor.matmul(
                                ps1[f][:, :nw],
                                lhsT=wma[:, k, f * 128:(f + 1) * 128],
                                rhs=rhs_t[:, k % dc, :nw],
                                start=(k == 0), stop=(k == 2 * dc - 1))
                    am = []
                    for f in range(dc):
                        a_ = apool.tile([128, 512], dt.bfloat16, name="am",
                                        tag="a1")
                        nc.scalar.activation(a_[:, :nw], ps1[f][:, :nw],
                                             AF.Relu, bias=bma[:, f:f + 1])
                        am.append(a_)
                    hstg = hstgp.tile([128, dc, 512], dt.bfloat16,
                                      name="hms", tag="hstg")
                    for f2 in range(dc):
                        p2 = pbig.tile([128, 512], dt.float32, name="psm2",
                                       tag="mlp")
                        for k2 in range(dc):
                            nc.tensor.matmul(
                                p2[:, :nw],
                                lhsT=wmb[:, k2, f2 * 128:(f2 + 1) * 128],
                                rhs=am[k2][:, :nw],
                                start=(k2 == 0), stop=(k2 == dc - 1))
                        nc.vector.tensor_scalar(
                            hstg[:, f2, :nw], p2[:, :nw],
                            bmb[:, f2:f2 + 1], None, ALU.add)
                    # table write for the next stage
                    dst = g_loc[2][:] if rnd == 1 else hf_loc[:]
                    table_write(hstg, g, gsz, dst)
                if rnd == 1:
                    nc.gpsimd.collective_compute(
                        "AllGather", ALU.bypass, replica_groups=rg,
                        ins=[g_loc[2][:]], outs=[g_full[2][:]])

            # =========== Phase 3: readout (push + ReduceScatter)
            parts = {}
            for i in (1, 2):
                parts[i] = dpool.tile([ncores * bpad, dim], dt.bfloat16,
                                      name=f"part{i}", tag=f"part{i}")
            for g in range(n_rogr):
                gsz = min(RO_GROUP, btg - g * RO_GROUP)
                for i in (1, 2):
                    nb = ro[i]["prep"]["nb"]
                    idxt = idxpool.tile([128, nb_max * 8], dt.int16,
                                        name="idxtr", tag="idx")
                    nc.sync.dma_start(idxt[:, :RO_GROUP * nb * 8],
                                      idxr_in[i][g])
                    segt = segpool.tile([128, nb_max, 128], dt.float8e4,
                                        name="segtr", tag="seg")
                    nc.scalar.dma_start(segt[:, :RO_GROUP * nb, :],
                                        segr_in[i][g])
                    ed = edpool.tile([128, nb_max, dim], GDT,
                                     name="edr", tag="ed")
                    q = next_q()
                    nc.gpsimd.reg_load(
                        cregs[q], cnts_sb[0:1, cnt_col_ro(i, g):
                                          cnt_col_ro(i, g) + 1])
                    nc.gpsimd.dma_gather(
                        ed[:, 0:RO_GROUP * nb, :], hf_loc[:],
                        idxt[:, 0:RO_GROUP * nb * 8],
                        RO_GROUP * nb * 128, cregs[q], dim,
                        single_packet=SINGLE_PACKET, queue_num=q)
                    for j in range(gsz):
                        tT = g * RO_GROUP + j
                        agg = paggp.tile([128, dim], dt.float32, name="aggr",
                                         tag="agg")
                        npair = nb // 2
                        for bp in range(npair):
                            off = j * nb + 2 * bp
                            nc.tensor.matmul(
                                agg[:],
                                lhsT=segt[:, off:off + 2, :],
                                rhs=ed[:, off:off + 2, :],
                                start=(bp == 0), stop=(bp == npair - 1),
                                perf_mode=DR)
                        aggs = mpool.tile([128, dim], dt.bfloat16,
                                          name="aggsr", tag="aggs")
                        nc.scalar.activation(aggs[:], agg[:], AF.Copy,
                                             scale=POST_SCALE)
                        nc.sync.dma_start(
                            parts[i][tT * 128:(tT + 1) * 128, :], aggs[:])
            rsh = {}
            for i in (1, 2):
                rs = dpool.tile([bpad, dim], dt.bfloat16, name=f"rsh{i}",
                                tag=f"rsh{i}")
                nc.gpsimd.collective_compute(
                    "ReduceScatter", ALU.add, replica_groups=rg,
                    ins=[parts[i][:]], outs=[rs[:]])
                rsh[i] = rs

            # transpose RS shards to feature-major rcat [128, 2*dc, bpad]
            rcat = wpool.tile([128, 2 * dc, bpad], dt.bfloat16, name="rcat",
                              tag="rcat")
            for i in (1, 2):
                for tb in range(bt):
                    rt = mpool.tile([128, dim], dt.bfloat16, name="rt",
                                    tag="rt")
                    nc.sync.dma_start(rt[:],
                                      rsh[i][tb * 128:(tb + 1) * 128, :])
                    for f in range(dc):
                        tp = pcnv.tile([128, 128], dt.bfloat16, name="tpr",
                                       tag="cnv")
                        nc.tensor.transpose(
                            tp[:], rt[:, f * 128:(f + 1) * 128],
                            wsb["ident16"][:])
                        nc.vector.tensor_copy(
                            rcat[:, (i - 1) * dc + f,
                                 tb * 128:(tb + 1) * 128], tp[:])

            # ---- final MLP + log_softmax
            logitsT = wpool.tile([128, bpad], dt.float32, name="logitsT",
                                 tag="logitsT")
            for (n0, nw) in _nchunks(bpad, 512):
                ps1 = [pbig.tile([128, 512], dt.float32, name="psf1",
                                 tag="mlp") for f in range(dc)]
                for k in range(2 * dc):
                    for f in range(dc):
                        nc.tensor.matmul(
                            ps1[f][:, :nw],
                            lhsT=wsb["wfa"][:, k, f * 128:(f + 1) * 128],
                            rhs=rcat[:, k, n0:n0 + nw],
                            start=(k == 0), stop=(k == 2 * dc - 1))
                af = []
                for f in range(dc):
                    a_ = apool.tile([128, 512], dt.bfloat16, name="af",
                                    tag="a1")
                    nc.scalar.activation(a_[:, :nw], ps1[f][:, :nw], AF.Relu,
                                         bias=wsb["bfa"][:, f:f + 1])
                    af.append(a_)
                pl = pbig.tile([128, 512], dt.float32, name="psl", tag="mlp")
                for k2 in range(dc):
                    nc.tensor.matmul(
                        pl[:ncls, :nw],
                        lhsT=wsb["wfb"][:, k2, :ncls],
                        rhs=af[k2][:, :nw],
                        start=(k2 == 0), stop=(k2 == dc - 1))
                nc.vector.tensor_scalar(
                    logitsT[:ncls, n0:n0 + nw], pl[:ncls, :nw],
                    wsb["bfb"][:ncls, 0:1], None, ALU.add)

            for tb in range(bt):
                ltp = pcnv.tile([128, 128], dt.float32, name="ltp",
                               tag="cnv")
                nc.tensor.transpose(
                    ltp[:], logitsT[:, tb * 128:(tb + 1) * 128],
                    wsb["ident32"][:])
                mx = mpool.tile([128, 1], dt.float32, name="mx", tag="mx")
                nc.vector.tensor_reduce(mx[:], ltp[:, :ncls],
                                        mybir.AxisListType.X, ALU.max)
                z = mpool.tile([128, ncls], dt.float32, name="z", tag="z")
                nc.vector.tensor_scalar(z[:], ltp[:, :ncls], mx[:, 0:1], None,
                                        ALU.subtract)
                ez = mpool.tile([128, ncls], dt.float32, name="ez", tag="z")
                nc.scalar.activation(ez[:], z[:], AF.Exp)
                sm = mpool.tile([128, 1], dt.float32, name="sm", tag="mx")
                nc.vector.tensor_reduce(sm[:], ez[:], mybir.AxisListType.X,
                                        ALU.add)
                ls = mpool.tile([128, 1], dt.float32, name="ls", tag="mx")
                nc.scalar.activation(ls[:], sm[:], AF.Ln)
                o = mpool.tile([128, ncls], dt.float32, name="o", tag="z")
                nc.vector.tensor_scalar(o[:], z[:], ls[:, 0:1], None,
                                        ALU.subtract)
                nc.sync.dma_start(out_dram[tb * 128:(tb + 1) * 128, :], o[:])

    nc.compile()
    return nc


def build_in_maps(cfg):
    in_maps = []
    for p in range(cfg["ncores"]):
        m = dict(
            xTc=cfg["xTc"][p],
            seg1=cfg["rel"][1]["prep"]["seg"][p],
            idx1=cfg["rel"][1]["prep"]["idx"][p],
            seg2=cfg["rel"][2]["prep"]["seg"][p],
            idx2=cfg["rel"][2]["prep"]["idx"][p],
            dinv2nm1=cfg["rel"][1]["dinv2_nm"][p],
            dinv2nm2=cfg["rel"][2]["dinv2_nm"][p],
            segr1=cfg["ro"][1]["prep"]["seg"][p],
            idxr1=cfg["ro"][1]["prep"]["idx"][p],
            segr2=cfg["ro"][2]["prep"]["seg"][p],
            idxr2=cfg["ro"][2]["prep"]["idx"][p],
            cnts=cfg["cnts"][p],
        )
        m.update({k: v for k, v in cfg["w"].items()})
        in_maps.append(m)
    return in_maps


_CACHE = {}


def kernel(**inputs) -> np.ndarray:
    cfg = host_prep(inputs)
    key = (
        cfg["t_nodes"], cfg["f_in"], cfg["dim"], cfg["ncls"], cfg["n_bins"],
        tuple((cfg["rel"][r]["prep"]["nb_lo"], cfg["rel"][r]["prep"]["nb_hi"])
              for r in (1, 2)),
        tuple(cfg["ro"][i]["prep"]["nb"] for i in (1, 2)),
    )
    if key not in _CACHE:
        _CACHE[key] = build_program(cfg)
    nc = _CACHE[key]

    from concourse.bass_utils import run_bass_kernel_spmd

    in_maps = build_in_maps(cfg)
    res = run_bass_kernel_spmd(nc, in_maps, list(range(cfg["ncores"])))
    outs = [res.results[p]["out"][: cfg["bpc"]] for p in range(cfg["ncores"])]
    return np.ascontiguousarray(np.concatenate(outs, axis=0), np.float32)
